# revision 23
# baseline (speedup 1.0000x reference)
"""MoE (noisy top-2-of-8 gating) Trainium2 kernel — v2.

Strategy: data-parallel over tokens (1024/core on 8 cores). Host computes
routing structure only; all FLOPs (gating values, expert MLPs, combine) run
on device. Tokens are permuted into 8 expert segments (experts sorted by
descending count; per-segment capacity = max count over cores so one SPMD
program serves all cores).

v2 changes vs the original (218994ns) kernel:
  * All DRAM operands are host-relaid so each logical load is 1-4 large
    DMAs (2KB+/partition lines): 8 gate-feature tile loads, 4 chunks per
    expert for fc1/fc2 weights, 1 per expert for x segments. HWDGE
    descriptor-generation drops from 218 ops (136us serialized) to ~90.
  * fc1 psums are built two h-chunks per 2-bank PSUM tile and drained by
    ONE gelu per pair: the ~390ns fixed ACT cost amortizes over 2x the
    elements, phase B stops being ACT-bound (78us -> ~42us of ACT).
  * fc2 exp'd output tiles are scattered in PAIRS (one indirect DMA per
    two 128-row chunks) and combine reads / y stores move off the Pool
    engine (SP + ACT issue them): the GPSIMD descriptor-gen serialization
    that dominated the old 22us tail shrinks to ~13 ops.
  * y is stored bf16 (host upcasts); adds ~2e-4 rel err, halves tail
    store time.

The Exp/Ln table-set chooser in bass is naive (picks the first set with
the function, thrashing between exp_and_others/natural_log on every
transition) — we point both at natural_log_exp_and_others (which
genuinely contains both) so each phase boundary is one table load.
"""

import numpy as np
import ml_dtypes

import concourse.bacc as bacc
import concourse.bass as bass
import concourse.mybir as mybir
import concourse.tile as tile
import concourse.hw_specs as hw_specs
from concourse.bass_utils import run_bass_kernel_spmd

BF16 = mybir.dt.bfloat16
FP32 = mybir.dt.float32
AF = mybir.ActivationFunctionType

N, D, H, E, TOPK = 8192, 512, 2048, 8, 2
NC = 8
NS = N // NC          # tokens per core
P = 128
NTT = NS // P         # token tiles per core (8)
DC = D // P           # d chunks (4)
HC = H // P           # hidden chunks (16)
HC2 = HC // 2         # fused gelu pairs (8)
FC = (2 * D) // P     # gate feature chunks (8)
NQ = 4                # SWDGE queues (hardware max)
WCH = 4               # weight load chunks per expert

_nc_cache: dict = {}
_act_tables_patched = [False]


def _patch_act_tables(arch: str):
    """Make Exp and Ln both resolve to natural_log_exp_and_others so the
    act-table fixpoint emits one load per phase instead of one per
    Exp<->Ln transition."""
    tabs = hw_specs.get_activation_tables(arch)
    if "natural_log_exp_and_others" in tabs:
        both = tabs["natural_log_exp_and_others"]
        if AF.Exp in both and AF.Ln in both:
            tabs["exp_and_others"].discard(AF.Exp)
            tabs["natural_log"].discard(AF.Ln)
    _act_tables_patched[0] = True


def _scatter_groups(ntts):
    """Pair fc2 output chunks within each expert: list of (k, ch0, ncols)."""
    groups = []
    ch = 0
    for k, n in enumerate(ntts):
        t = 0
        while t < n:
            w = 2 if t + 1 < n else 1
            groups.append((k, ch + t, w))
            t += w
        ch += n
    return groups


def _build_nc(caps, rsegs=(7,) * 8, reps=1, gelu_sub=False, timing=False,
              skip=(), wbufs=4, ps2=2, use_b2=True, unroll=8, dbg=False):
    """Build the SPMD Bass program for per-segment capacities `caps`."""
    gelu_af = AF.Tanh if gelu_sub else AF.Gelu
    caps = tuple(int(c) for c in caps)
    R = sum(caps)
    offs = np.concatenate([[0], np.cumsum(caps)]).astype(int)
    ntts = [(c + P - 1) // P for c in caps]
    NCH = sum(ntts)              # fc2 output tiles == scatter chunk columns
    TROWS = 2 * NS + P           # table1 | table2 | dump rows
    groups = _scatter_groups(ntts)

    nc = bacc.Bacc("TRN2", target_bir_lowering=False, debug=False,
                   num_swdge_queues=NQ)
    if not _act_tables_patched[0]:
        _patch_act_tables(nc.m.arch)

    if timing:
        def param(name, shape, dtype):
            return nc.dram_tensor(name, shape, dtype)
        dummy_d = nc.declare_dram_parameter("tdin", [1, 4], FP32, isOutput=False)
        y_d = nc.dram_tensor("y", [NS, D], BF16)
        yo_d = nc.declare_dram_parameter("yo", [1, 4], FP32, isOutput=True)
    else:
        def param(name, shape, dtype):
            return nc.declare_dram_parameter(name, shape, dtype, isOutput=False)
        y_d = nc.declare_dram_parameter("y", [NS, D], BF16, isOutput=True)

    xl_d = param("xl", [P, DC * R], BF16)
    gft_d = param("gft", [P, NTT * FC * P], BF16)
    nst_d = param("nst", [P, NTT * E], FP32)
    wgwn_d = param("wgwn", [P, FC * 2 * E], BF16)
    w1t_d = param("w1t", [E, P, DC * H], BF16)
    w2t_d = param("w2t", [E, P, HC * D], BF16)
    b2_d = param("b2", [E, D], BF16)
    sidx_d = param("sidx", [P, NCH], mybir.dt.int32)

    with tile.TileContext(nc) as tc:
        with (
            tc.tile_pool(name="const", bufs=1) as constp,
            tc.tile_pool(name="gate", bufs=1) as gatep,
            tc.tile_pool(name="hall", bufs=1) as hallp,
            tc.tile_pool(name="xpool", bufs=1) as xp,
            tc.tile_pool(name="wpool", bufs=wbufs) as wp,
            tc.tile_pool(name="apool", bufs=3) as ap_,
            tc.tile_pool(name="cpool", bufs=4) as cp,
            tc.tile_pool(name="cspool", bufs=3) as cps_,
            tc.tile_pool(name="psum", bufs=1, space="PSUM") as pp,
            tc.tile_pool(name="dram", bufs=1, space="DRAM") as dp,
        ):
            ones1 = constp.tile([1, P], BF16)
            nc.vector.memset(ones1[:], 1.0)
            dummy4 = constp.tile([1, 4], FP32)
            nc.vector.memset(dummy4[:], 0.0)

            def load_w(which, d_, k):
                wsb = wp.tile([P, DC * H], BF16, tag="w")
                step = (DC * H) // WCH
                for c in range(WCH if "wdma" not in skip else 1):
                    nc.sync.dma_start(
                        out=wsb[:, c * step : (c + 1) * step],
                        in_=d_[k, :, c * step : (c + 1) * step],
                    )
                if which == 2 and use_b2:
                    b2sb = wp.tile([1, D], BF16, tag="b2")
                    nc.sync.dma_start(out=b2sb[:], in_=b2_d[k][None, :])
                    return wsb, b2sb
                return wsb, None

            def load_x(k):
                cap = caps[k]
                o0 = int(offs[k])
                xk = xp.tile([P, DC * cap], BF16, tag=f"x{k}")
                nc.sync.dma_start(out=xk[:], in_=xl_d[:, DC * o0 : DC * (o0 + cap)])
                return xk

            def body(_i=None):
                # ---------- persistent loads (gating first) ----------
                gsb = gatep.tile([P, NTT * FC * P], BF16, tag="gsb", bufs=2)
                for t in range(NTT):
                    nc.sync.dma_start(
                        out=gsb[:, t * FC * P : (t + 1) * FC * P],
                        in_=gft_d[:, t * FC * P : (t + 1) * FC * P],
                    )
                wg2sb = gatep.tile([P, FC * 2 * E], BF16, tag="wg2sb")
                nc.sync.dma_start(out=wg2sb[:], in_=wgwn_d[:])
                nssb = gatep.tile([P, NTT * E], FP32, tag="nssb")
                nc.sync.dma_start(out=nssb[:], in_=nst_d[:])
                sidxsb = gatep.tile([P, NCH], mybir.dt.int32, tag="sidxsb")
                if timing:
                    nc.vector.memset(sidxsb[:], 0)
                else:
                    nc.sync.dma_start(out=sidxsb[:], in_=sidx_d[:])
                # x0 before the bulk of w1(0): the first fc1 psum pair needs
                # only w1 chunk 0 + x0, and x0 is the smaller transfer
                x_pre = {0: load_x(0)}
                w_pre = {0: load_w(1, w1t_d, 0)}
                x_pre[1] = load_x(1)

                # ---------- phase A: gating, token-major ----------
                g1sb = gatep.tile([P, NTT], FP32, tag="g1")
                g2sb = gatep.tile([P, NTT], FP32, tag="g2")
                if "gate" in skip:
                    nc.vector.memset(g1sb[:], 0.5)
                    nc.vector.memset(g2sb[:], 0.5)
                else:
                    W2E = 2 * E
                    clsb = gatep.tile([P, NTT * E], FP32, tag="clsb")
                    nssp = gatep.tile([P, NTT * E], FP32, tag="nssp")
                    for t in range(NTT):
                        gpsw = pp.tile([P, 2, 512], FP32, tag="fc1_ps", bufs=3)
                        gps = gpsw[:, 0, :W2E]
                        for c in range(FC):
                            nc.tensor.matmul(
                                gps[:],
                                lhsT=gsb[:, t * FC * P + c * P : t * FC * P + (c + 1) * P],
                                rhs=wg2sb[:, c * W2E : (c + 1) * W2E],
                                start=(c == 0),
                                stop=(c == FC - 1),
                            )
                        nc.vector.tensor_copy(
                            clsb[:, t * E : (t + 1) * E], gps[:, :E]
                        )
                        nc.vector.tensor_copy(
                            nssp[:, t * E : (t + 1) * E], gps[:, E:]
                        )
                    # stddev = softplus(noise logits) + 1e-2
                    nc.scalar.activation(nssp[:], nssp[:], AF.Exp)
                    nc.vector.tensor_scalar_add(nssp[:], nssp[:], 1.0)
                    nc.scalar.activation(nssp[:], nssp[:], AF.Ln)
                    nc.vector.tensor_scalar_add(nssp[:], nssp[:], 1e-2)
                    # logits = clean + noise * stddev; exp once — top-2 of
                    # exp(logits) == exp(top-2), so the gate softmax is pure
                    # DVE math after this
                    nc.vector.tensor_mul(nssp[:], nssp[:], nssb[:])
                    nc.vector.tensor_add(clsb[:], clsb[:], nssp[:])
                    nc.scalar.activation(clsb[:], clsb[:], AF.Exp)
                    # fence emitted BEFORE the g1/g2 math so gelus unblock as
                    # soon as the exp lands
                    gfence = gatep.tile([P, 1], FP32, tag="gfence")
                    nc.vector.tensor_scalar_mul(gfence[:], clsb[:, 0:1], 0.0)
                    mx8 = gatep.tile([P, NTT * E], FP32, tag="mx8")
                    t1g = gatep.tile([P, NTT], FP32, tag="t1g")
                    e1a = gatep.tile([P, NTT], FP32, tag="e1a")
                    e2a = gatep.tile([P, NTT], FP32, tag="e2a")
                    for t in range(NTT):
                        nc.vector.max(
                            out=mx8[:, t * E : (t + 1) * E],
                            in_=clsb[:, t * E : (t + 1) * E],
                        )
                        nc.vector.tensor_copy(
                            e1a[:, t : t + 1], mx8[:, t * E : t * E + 1]
                        )
                        nc.vector.tensor_copy(
                            e2a[:, t : t + 1], mx8[:, t * E + 1 : t * E + 2]
                        )
                    nc.vector.tensor_add(t1g[:], e1a[:], e2a[:])
                    nc.vector.reciprocal(t1g[:], t1g[:])
                    nc.vector.tensor_mul(g1sb[:], e1a[:], t1g[:])
                    # r = g2/g1 = e2/e1 for the fused combine (b1 + r*b2)
                    nc.vector.reciprocal(e1a[:], e1a[:])
                    nc.vector.tensor_mul(g2sb[:], e2a[:], e1a[:])

                # ---------- phase B: fc1 + gelu (fused pairs) ----------
                if "gate" in skip:
                    gfence = gatep.tile([P, 1], FP32, tag="gfence")
                    nc.vector.tensor_scalar_mul(gfence[:], g2sb[:, 0:1], 0.0)
                hall = []
                for k in range(E):
                    if k + 2 < E:
                        x_pre[k + 2] = load_x(k + 2)
                    if k + 1 < E:
                        w_pre[k + 1] = load_w(1, w1t_d, k + 1)
                    if k == E - 2:
                        w2_pre = {0: load_w(2, w2t_d, 0)}
                    if k == E - 1:
                        w2_pre[1] = load_w(2, w2t_d, 1)
                    w1sb, _ = w_pre.pop(k)
                    xk = x_pre.pop(k)
                    cap = caps[k]
                    hsb = hallp.tile([P, HC * cap], BF16, tag=f"h{k}")
                    for hp in range(HC2 if "fc1" not in skip else 1):
                        p2 = pp.tile([P, 2, 512], FP32, tag="fc1_ps", bufs=3)
                        for hh in range(2):
                            h = 2 * hp + hh
                            for d_ in range(DC):
                                nc.tensor.matmul(
                                    p2[:, hh, :cap],
                                    lhsT=w1sb[:, h * 512 + d_ * P : h * 512 + (d_ + 1) * P],
                                    rhs=xk[:, d_ * cap : (d_ + 1) * cap],
                                    start=(d_ == 0),
                                    stop=(d_ == DC - 1),
                                )
                        if "gelu" in skip:
                            nc.vector.tensor_copy(
                                hsb[:, 2 * hp * cap : (2 * hp + 2) * cap],
                                p2[:, :, :cap],
                            )
                        else:
                            nc.scalar.activation(
                                hsb[:, 2 * hp * cap : (2 * hp + 2) * cap].rearrange(
                                    "p (two c) -> p two c", two=2
                                ),
                                p2[:, :, :cap],
                                gelu_af,
                                bias=gfence[:, 0:1],
                            )
                    hall.append(hsb)

                # ---------- phase C: fc2 + exp + scatter + combine ----------
                lastc = HC * caps[E - 1]
                fence = gatep.tile([P, 1], FP32, tag="fence")
                nc.vector.tensor_scalar_mul(
                    fence[:], hall[E - 1][:, lastc - 1 : lastc], 0.0
                )
                if dbg:
                    tab = nc.declare_dram_parameter(
                        "tabd", [TROWS, D], BF16, isOutput=True
                    )
                    gdump = nc.declare_dram_parameter(
                        "gdump", [P, 2 * NTT], FP32, isOutput=True
                    )
                    nc.sync.dma_start(out=gdump[:, :NTT], in_=g1sb[:])
                    nc.sync.dma_start(out=gdump[:, NTT:], in_=g2sb[:])
                    hdump = nc.declare_dram_parameter(
                        "hdump", [P, HC * caps[0]], BF16, isOutput=True
                    )
                    nc.sync.dma_start(out=hdump[:], in_=hall[0][:])
                else:
                    tab = dp.tile([TROWS, D], BF16, tag="a_tab")

                comb_reads = []
                q_latest = {}

                def emit_combine(t0, L):
                    # combine L (1-2) adjacent token tiles. The table is
                    # interleaved (row = tile*256 + half*128 + p) so ONE read
                    # fetches both experts' rows; the per-tile math is a
                    # single fused DVE op t = b2*r + b1 (r = g2/g1, stored in
                    # g2sb) and y = Ln(t * g1) via the activation scale.
                    bg = cp.tile([P, L * 2 * D], BF16, tag="bg", bufs=3)
                    snap = dict(q_latest)
                    r1 = nc.sync.dma_start(
                        out=bg[:].rearrange("p (l h j) -> p l h j", l=L, h=2),
                        in_=tab[t0 * 2 * P : (t0 + L) * 2 * P, :].rearrange(
                            "(l h p) j -> p l h j", l=L, h=2
                        ),
                    )
                    comb_reads.append((r1, snap))
                    sv = cps_.tile([P, L * D], BF16, tag="sv", bufs=2)
                    yv = cps_.tile([P, L * D], BF16, tag="yv", bufs=2)
                    for l in range(L):
                        t = t0 + l
                        nc.vector.scalar_tensor_tensor(
                            out=sv[:, l * D : (l + 1) * D],
                            in0=bg[:, (2 * l + 1) * D : (2 * l + 2) * D],
                            scalar=g2sb[:, t : t + 1],
                            in1=bg[:, 2 * l * D : (2 * l + 1) * D],
                            op0=mybir.AluOpType.mult,
                            op1=mybir.AluOpType.add,
                        )
                        nc.scalar.activation(
                            yv[:, l * D : (l + 1) * D],
                            sv[:, l * D : (l + 1) * D],
                            AF.Ln,
                            scale=g1sb[:, t : t + 1],
                        )
                    nc.scalar.dma_start(
                        out=y_d[t0 * P : (t0 + L) * P, :].rearrange(
                            "(l p) j -> p l j", l=L
                        ),
                        in_=yv[:].rearrange("p (l j) -> p l j", l=L),
                    )

                scatters = []
                ch = 0
                for k in range(E):
                    if k + 2 < E:
                        w2_pre[k + 2] = load_w(2, w2t_d, k + 2)
                    w2sb, b2sb = w2_pre.pop(k)
                    cap = caps[k]
                    hsb = hall[k]
                    for tt in range(ntts[k]):
                        m = min(P, cap - tt * P)
                        asb = ap_.tile([P, D], BF16, tag="a_sb")
                        ps2t = pp.tile([P, D], FP32, tag="fc2_ps", bufs=ps2)
                        nh = HC if "fc2" not in skip else 1
                        for h in range(nh):
                            nc.tensor.matmul(
                                ps2t[:m],
                                lhsT=hsb[:, h * cap + tt * P : h * cap + tt * P + m],
                                rhs=w2sb[:, h * D : (h + 1) * D],
                                start=(h == 0),
                                stop=(h == nh - 1 and not use_b2),
                            )
                        if use_b2:
                            nc.tensor.matmul(
                                ps2t[:m],
                                lhsT=ones1[:, :m],
                                rhs=b2sb[:],
                                start=False,
                                stop=True,
                            )
                        nc.scalar.activation(
                            asb[:m], ps2t[:m], AF.Exp, bias=fence[:m, 0:1]
                        )
                        if "scatter" not in skip:
                            mm = max(m, 2)
                            si = nc.gpsimd.indirect_dma_start(
                                out=tab[:],
                                out_offset=bass.IndirectOffsetOnAxis(
                                    ap=sidxsb[:mm, ch : ch + 1], axis=0
                                ),
                                in_=asb[:mm],
                                in_offset=None,
                            )
                            si.ins.queue = f"qPoolDynamic{(ch % NQ) or ''}"
                            q_latest[ch % NQ] = si
                            scatters.append(si)
                        ch += 1
                    if "tail" not in skip:
                        t = 0
                        while t < NTT:
                            if rsegs[t] == k:
                                L = 2 if (t + 1 < NTT and rsegs[t + 1] == k) else 1
                                emit_combine(t, L)
                                t += L
                            else:
                                t += 1
                # scatter destination rows are disjoint (injective dest map):
                # strip false WAW/WAR deps from the tracker's full-range AP so
                # scatters pipeline; combine reads then explicitly wait on the
                # latest scatter of every queue emitted before them.
                false_dep = {i.ins.name for i in scatters} | {
                    r.ins.name for r, _ in comb_reads
                }
                for si in scatters:
                    for nm in list(si.ins.sync_dependency_names()):
                        if nm in false_dep:
                            si.ins.try_remove_dependency(nm)
                dinfo = None
                for ri, snap in comb_reads:
                    have = set(ri.ins.sync_dependency_names())
                    if dinfo is None and have:
                        dinfo = ri.ins.get_dependency_info(next(iter(have)))
                    for si in snap.values():
                        if si.ins.name not in have:
                            ri.ins.add_dependency(si.ins.name, dinfo)

            if reps > 1:
                U = unroll
                while reps % U:
                    U -= 1
                with tc.For_i(0, reps // U, 1, staggered_reset=True):
                    for _u in range(U):
                        body()
            else:
                body()
            if timing:
                nc.sync.dma_start(out=yo_d[:], in_=dummy4[:])

    nc.compile()
    return nc


def _route(gate_feat, noise, w_gate, w_noise):
    """Host-side routing structure (fp32 numpy, matches jax top-k selection)."""
    clean = gate_feat @ w_gate
    stddev = np.logaddexp(gate_feat @ w_noise, 0.0) + np.float32(1e-2)
    logits = clean.astype(np.float32) + noise * stddev.astype(np.float32)
    top2 = np.argsort(-logits, axis=1, kind="stable")[:, :TOPK].astype(np.int32)
    return top2


def _prepare(x, gate_feat, noise, w_gate, w_noise, fc1_w, fc1_b, fc2_w, fc2_b):
    x = np.ascontiguousarray(x, dtype=np.float32)
    gate_feat = np.ascontiguousarray(gate_feat, dtype=np.float32)
    noise = np.ascontiguousarray(noise, dtype=np.float32)

    top2 = _route(gate_feat, noise, w_gate, w_noise)

    bf = ml_dtypes.bfloat16
    # w1l[e, p, h*512 + d*128 + q] = fc1_w[e, h*128+q, d*128+p]
    w1l_all = np.ascontiguousarray(
        fc1_w.reshape(E, HC, P, DC, P).transpose(0, 4, 1, 3, 2).reshape(E, P, DC * H)
    ).astype(bf)
    # w2l[e, p, h*512 + j] = fc2_w[e, j, h*128+p]
    w2l_all = np.ascontiguousarray(
        fc2_w.reshape(E, D, HC, P).transpose(0, 3, 2, 1).reshape(E, P, HC * D)
    ).astype(bf)
    b2_all = np.ascontiguousarray(fc2_b).astype(bf)
    wgwn = np.hstack([w_gate, w_noise]).astype(np.float32)
    # wgwn2[p, c*16+j] = wgwn[c*128+p, j]
    wgwn2 = np.ascontiguousarray(
        wgwn.reshape(FC, P, 2 * E).transpose(1, 0, 2).reshape(P, FC * 2 * E)
    ).astype(bf)

    core_meta = []
    for c in range(NC):
        t2 = top2[c * NS : (c + 1) * NS]
        cnt = np.bincount(t2.ravel(), minlength=E)
        order = np.argsort(-cnt, kind="stable").astype(np.int32)
        seg_of_expert = np.empty(E, dtype=np.int64)
        seg_of_expert[order] = np.arange(E)
        pair_seg = seg_of_expert[t2.ravel()]
        sort_idx = np.argsort(pair_seg, kind="stable")
        seg_counts = cnt[order]
        core_meta.append((t2, order, pair_seg, sort_idx, seg_counts))

    caps = np.max(np.stack([m[4] for m in core_meta]), axis=0)
    offs = np.concatenate([[0], np.cumsum(caps)]).astype(np.int64)
    R = int(offs[-1])
    ntts = [(int(c) + P - 1) // P for c in caps]
    NCH = sum(ntts)

    in_maps = []
    perms = []
    rsegs_cores = []
    for c in range(NC):
        t2, order, pair_seg, sort_idx, seg_counts = core_meta[c]
        pos_in_seg = np.arange(2 * NS) - np.concatenate([[0], np.cumsum(seg_counts)])[pair_seg[sort_idx]]
        rows_sorted = offs[pair_seg[sort_idx]] + pos_in_seg
        rows_of_pair = np.empty(2 * NS, dtype=np.int64)
        rows_of_pair[sort_idx] = rows_sorted

        ready = np.maximum(pair_seg[0::2], pair_seg[1::2])
        perm = np.argsort(ready, kind="stable")
        inv_perm = np.empty(NS, dtype=np.int64)
        inv_perm[perm] = np.arange(NS)
        rseg_core = ready[perm].reshape(NTT, P).max(axis=1)

        # interleaved table: row = tile*256 + half*128 + p
        dest = np.empty(R + P, dtype=np.int32)
        dest[:] = 2 * NS + (np.arange(R + P) % P)
        base = (inv_perm // P) * (2 * P) + (inv_perm % P)
        dest[rows_of_pair[0::2]] = base
        dest[rows_of_pair[1::2]] = base + P
        sidx = np.zeros((P, NCH), dtype=np.int32)
        chv = 0
        for k in range(E):
            for tt in range(ntts[k]):
                s = int(offs[k]) + tt * P
                sidx[:, chv] = dest[s : s + P]
                # rows past this segment's capacity would alias the NEXT
                # segment's dest entries — route them to the dump region
                # (scatters run concurrently, so aliasing is a race)
                m = min(P, int(caps[k]) - tt * P)
                if m < P:
                    sidx[m:, chv] = 2 * NS + np.arange(m, P)
                chv += 1

        tok_sorted = sort_idx // 2
        cols = np.zeros(R, dtype=np.int64)
        for k in range(E):
            s0 = int(np.concatenate([[0], np.cumsum(seg_counts)])[k])
            cnt_k = int(seg_counts[k])
            cols[offs[k] : offs[k] + cnt_k] = tok_sorted[s0 : s0 + cnt_k]
        x_loc = x[c * NS : (c + 1) * NS]
        # xl[p, DC*offs[k] + d*cap_k + t] = x_loc[cols_k[t], d*128+p]
        xl = np.zeros((P, DC * R), dtype=bf)
        for k in range(E):
            o0, cap_k = int(offs[k]), int(caps[k])
            seg = x_loc[cols[o0 : o0 + cap_k]]           # [cap, D]
            xl[:, DC * o0 : DC * (o0 + cap_k)] = (
                seg.reshape(cap_k, DC, P).transpose(2, 1, 0).reshape(P, DC * cap_k)
            )

        gf_loc = gate_feat[c * NS : (c + 1) * NS]
        # gft[p, t*1024 + c*128 + q] = gf_loc[perm[t*128+q], c*128+p]
        gfp = gf_loc[perm]                                # [NS, 2D]
        gft = np.ascontiguousarray(
            gfp.reshape(NTT, P, FC, P).transpose(3, 0, 2, 1).reshape(P, NTT * FC * P)
        ).astype(bf)

        ns_loc = noise[c * NS : (c + 1) * NS]
        nst = np.ascontiguousarray(
            ns_loc[perm].reshape(NTT, P, E).transpose(1, 0, 2).reshape(P, NTT * E)
        ).astype(np.float32)
        in_maps.append({
            "xl": np.ascontiguousarray(xl),
            "gft": gft,
            "nst": nst,
            "wgwn": wgwn2,
            "w1t": np.ascontiguousarray(w1l_all[order]),
            "w2t": np.ascontiguousarray(w2l_all[order]),
            "b2": np.ascontiguousarray(b2_all[order]),
            "sidx": sidx,
        })
        perms.append(perm)
        rsegs_cores.append(rseg_core)

    rsegs = tuple(int(v) for v in np.max(np.stack(rsegs_cores), axis=0))
    return caps, rsegs, perms, in_maps


def kernel(x, gate_feat, noise, w_gate, w_noise, fc1_w, fc1_b, fc2_w, fc2_b,
           _reps=1):
    caps, rsegs, perms, in_maps = _prepare(
        x, gate_feat, noise, w_gate, w_noise, fc1_w, fc1_b, fc2_w, fc2_b
    )
    use_b2 = bool(np.any(np.asarray(fc2_b)))
    key = (tuple(int(v) for v in caps), rsegs, int(_reps), use_b2)
    if key not in _nc_cache:
        _nc_cache[key] = _build_nc(caps, rsegs, reps=_reps, use_b2=use_b2)
    nc = _nc_cache[key]
    try:
        res = run_bass_kernel_spmd(nc, in_maps, core_ids=list(range(NC)))
    except Exception:
        res = run_bass_kernel_spmd(nc, in_maps, core_ids=list(range(NC)))
    y = np.empty((N, D), np.float32)
    for c in range(NC):
        y[c * NS : (c + 1) * NS][perms[c]] = res.results[c]["y"].astype(np.float32)
    return y


# revision 29
# speedup vs baseline: 1.0154x; 1.0154x over previous
"""MoE (noisy top-2-of-8 gating) Trainium2 kernel — v2.

Strategy: data-parallel over tokens (1024/core on 8 cores). Host computes
routing structure only; all FLOPs (gating values, expert MLPs, combine) run
on device. Tokens are permuted into 8 expert segments (experts sorted by
descending count; per-segment capacity = max count over cores so one SPMD
program serves all cores).

v2 changes vs the original (218994ns) kernel:
  * All DRAM operands are host-relaid so each logical load is 1-4 large
    DMAs (2KB+/partition lines): 8 gate-feature tile loads, 4 chunks per
    expert for fc1/fc2 weights, 1 per expert for x segments. HWDGE
    descriptor-generation drops from 218 ops (136us serialized) to ~90.
  * fc1 psums are built two h-chunks per 2-bank PSUM tile and drained by
    ONE gelu per pair: the ~390ns fixed ACT cost amortizes over 2x the
    elements, phase B stops being ACT-bound (78us -> ~53us of ACT).
  * The scatter table is interleaved (row = tile*256 + half*128 + p) so
    each combine group (1-2 token tiles sharing a readiness segment) is
    ONE strided read instead of 2-4; the combine math collapses to one
    fused DVE op per tile, t = b2*(g2/g1) + b1 via scalar_tensor_tensor,
    and y = Ln(t*g1) via the activation scale — the old mul/mul/add/Ln
    chain (4 ops, 2 round trips) becomes 2 ops.
  * Combine reads and y stores ride SP/ACT HWDGE queues, leaving the
    Pool engine only the 21 solo scatters. (Paired 2-column scatters
    compute wrong on HW — the DynamicAP index unroll order differs from
    CoreSim — so scatters stay one column per fc2 output tile.)
  * y is stored bf16 (host upcasts); adds ~2e-4 rel err, halves tail
    store time.

The Exp/Ln table-set chooser in bass is naive (picks the first set with
the function, thrashing between exp_and_others/natural_log on every
transition) — we point both at natural_log_exp_and_others (which
genuinely contains both) so each phase boundary is one table load.
"""

import numpy as np
import ml_dtypes

import concourse.bacc as bacc
import concourse.bass as bass
import concourse.mybir as mybir
import concourse.tile as tile
import concourse.hw_specs as hw_specs
from concourse.bass_utils import run_bass_kernel_spmd

BF16 = mybir.dt.bfloat16
FP32 = mybir.dt.float32
AF = mybir.ActivationFunctionType

N, D, H, E, TOPK = 8192, 512, 2048, 8, 2
NC = 8
NS = N // NC          # tokens per core
P = 128
NTT = NS // P         # token tiles per core (8)
DC = D // P           # d chunks (4)
HC = H // P           # hidden chunks (16)
HC2 = HC // 2         # fused gelu pairs (8)
FC = (2 * D) // P     # gate feature chunks (8)
NQ = 4                # SWDGE queues (hardware max)
WCH = 4               # weight load chunks per expert

_nc_cache: dict = {}
_act_tables_patched = [False]


def _patch_act_tables(arch: str):
    """Make Exp and Ln both resolve to natural_log_exp_and_others so the
    act-table fixpoint emits one load per phase instead of one per
    Exp<->Ln transition."""
    tabs = hw_specs.get_activation_tables(arch)
    if "natural_log_exp_and_others" in tabs:
        both = tabs["natural_log_exp_and_others"]
        if AF.Exp in both and AF.Ln in both:
            tabs["exp_and_others"].discard(AF.Exp)
            tabs["natural_log"].discard(AF.Ln)
    _act_tables_patched[0] = True


def _build_nc(caps, rsegs=(7,) * 8, reps=1, gelu_sub=False, timing=False,
              skip=(), wbufs=4, ps2=2, use_b2=True, use_b1=False, unroll=8,
              dbg=False, gsb_bufs=1):
    """Build the SPMD Bass program for per-segment capacities `caps`."""
    gelu_af = AF.Tanh if gelu_sub else AF.Gelu
    caps = tuple(int(c) for c in caps)
    R = sum(caps)
    offs = np.concatenate([[0], np.cumsum(caps)]).astype(int)
    ntts = [(c + P - 1) // P for c in caps]
    NCH = sum(ntts)              # fc2 output tiles == scatter chunk columns
    TROWS = 2 * NS + P           # interleaved token tables | dump rows

    nc = bacc.Bacc("TRN2", target_bir_lowering=False, debug=False,
                   num_swdge_queues=NQ)
    if not _act_tables_patched[0]:
        _patch_act_tables(nc.m.arch)

    if timing:
        def param(name, shape, dtype):
            return nc.dram_tensor(name, shape, dtype)
        dummy_d = nc.declare_dram_parameter("tdin", [1, 4], FP32, isOutput=False)
        y_d = nc.dram_tensor("y", [NS, D], BF16)
        yo_d = nc.declare_dram_parameter("yo", [1, 4], FP32, isOutput=True)
    else:
        def param(name, shape, dtype):
            return nc.declare_dram_parameter(name, shape, dtype, isOutput=False)
        y_d = nc.declare_dram_parameter("y", [NS, D], BF16, isOutput=True)

    xl_d = param("xl", [P, DC * R], BF16)
    gft_d = param("gft", [P, NTT * FC * P], BF16)
    nst_d = param("nst", [P, NTT * E], FP32)
    wgwn_d = param("wgwn", [P, FC * 2 * E], BF16)
    w1t_d = param("w1t", [E, P, DC * H], BF16)
    w2t_d = param("w2t", [E, P, HC * D], BF16)
    b2_d = param("b2", [E, D], BF16)
    b1_d = param("b1", [E, P, HC], FP32) if use_b1 else None
    sidx_d = param("sidx", [P, NCH], mybir.dt.int32)

    with tile.TileContext(nc) as tc:
        with (
            tc.tile_pool(name="const", bufs=1) as constp,
            tc.tile_pool(name="gate", bufs=1) as gatep,
            tc.tile_pool(name="hall", bufs=1) as hallp,
            tc.tile_pool(name="xpool", bufs=1) as xp,
            tc.tile_pool(name="wpool", bufs=wbufs) as wp,
            tc.tile_pool(name="apool", bufs=3) as ap_,
            tc.tile_pool(name="cpool", bufs=4) as cp,
            tc.tile_pool(name="cspool", bufs=3) as cps_,
            tc.tile_pool(name="psum", bufs=1, space="PSUM") as pp,
            tc.tile_pool(name="dram", bufs=1, space="DRAM") as dp,
        ):
            ones1 = constp.tile([1, P], BF16)
            nc.vector.memset(ones1[:], 1.0)
            dummy4 = constp.tile([1, 4], FP32)
            nc.vector.memset(dummy4[:], 0.0)

            def load_w(which, d_, k):
                wsb = wp.tile([P, DC * H], BF16, tag="w")
                step = (DC * H) // WCH
                for c in range(WCH if "wdma" not in skip else 1):
                    nc.sync.dma_start(
                        out=wsb[:, c * step : (c + 1) * step],
                        in_=d_[k, :, c * step : (c + 1) * step],
                    )
                if which == 2 and use_b2:
                    b2sb = wp.tile([1, D], BF16, tag="b2")
                    nc.sync.dma_start(out=b2sb[:], in_=b2_d[k][None, :])
                    return wsb, b2sb
                if which == 1 and use_b1:
                    b1sb = wp.tile([P, HC], FP32, tag="b1")
                    nc.sync.dma_start(out=b1sb[:], in_=b1_d[k])
                    return wsb, b1sb
                return wsb, None

            def load_x(k):
                cap = caps[k]
                o0 = int(offs[k])
                xk = xp.tile([P, DC * cap], BF16, tag=f"x{k}")
                nc.sync.dma_start(out=xk[:], in_=xl_d[:, DC * o0 : DC * (o0 + cap)])
                return xk

            def body(_i=None):
                # ---------- persistent loads (gating first) ----------
                gsb = gatep.tile([P, NTT * FC * P], BF16, tag="gsb",
                                 bufs=gsb_bufs)
                for t in range(NTT):
                    nc.sync.dma_start(
                        out=gsb[:, t * FC * P : (t + 1) * FC * P],
                        in_=gft_d[:, t * FC * P : (t + 1) * FC * P],
                    )
                wg2sb = gatep.tile([P, FC * 2 * E], BF16, tag="wg2sb")
                nc.sync.dma_start(out=wg2sb[:], in_=wgwn_d[:])
                nssb = gatep.tile([P, NTT * E], FP32, tag="nssb")
                nc.sync.dma_start(out=nssb[:], in_=nst_d[:])
                sidxsb = gatep.tile([P, NCH], mybir.dt.int32, tag="sidxsb")
                if timing:
                    nc.vector.memset(sidxsb[:], 0)
                else:
                    nc.sync.dma_start(out=sidxsb[:], in_=sidx_d[:])
                # x0 before the bulk of w1(0): the first fc1 psum pair needs
                # only w1 chunk 0 + x0, and x0 is the smaller transfer
                x_pre = {0: load_x(0)}
                w_pre = {0: load_w(1, w1t_d, 0)}
                x_pre[1] = load_x(1)

                # ---------- phase A: gating, token-major ----------
                g1sb = gatep.tile([P, NTT], FP32, tag="g1")
                g2sb = gatep.tile([P, NTT], FP32, tag="g2")
                if "gate" in skip:
                    nc.vector.memset(g1sb[:], 0.5)
                    nc.vector.memset(g2sb[:], 0.5)
                else:
                    W2E = 2 * E
                    clsb = gatep.tile([P, NTT * E], FP32, tag="clsb")
                    nssp = gatep.tile([P, NTT * E], FP32, tag="nssp")
                    for t in range(NTT):
                        gpsw = pp.tile([P, 2, 512], FP32, tag="fc1_ps", bufs=3)
                        gps = gpsw[:, 0, :W2E]
                        for c in range(FC):
                            nc.tensor.matmul(
                                gps[:],
                                lhsT=gsb[:, t * FC * P + c * P : t * FC * P + (c + 1) * P],
                                rhs=wg2sb[:, c * W2E : (c + 1) * W2E],
                                start=(c == 0),
                                stop=(c == FC - 1),
                            )
                        nc.vector.tensor_copy(
                            clsb[:, t * E : (t + 1) * E], gps[:, :E]
                        )
                        nc.vector.tensor_copy(
                            nssp[:, t * E : (t + 1) * E], gps[:, E:]
                        )
                    # stddev = softplus(noise logits) + 1e-2
                    nc.scalar.activation(nssp[:], nssp[:], AF.Exp)
                    nc.vector.tensor_scalar_add(nssp[:], nssp[:], 1.0)
                    nc.scalar.activation(nssp[:], nssp[:], AF.Ln)
                    nc.vector.tensor_scalar_add(nssp[:], nssp[:], 1e-2)
                    # logits = clean + noise * stddev; exp once — top-2 of
                    # exp(logits) == exp(top-2), so the gate softmax is pure
                    # DVE math after this
                    nc.vector.tensor_mul(nssp[:], nssp[:], nssb[:])
                    nc.vector.tensor_add(clsb[:], clsb[:], nssp[:])
                    nc.scalar.activation(clsb[:], clsb[:], AF.Exp)
                    # fence emitted BEFORE the g1/g2 math so gelus unblock as
                    # soon as the exp lands
                    gfence = gatep.tile([P, 1], FP32, tag="gfence")
                    nc.vector.tensor_scalar_mul(gfence[:], clsb[:, 0:1], 0.0)
                    mx8 = gatep.tile([P, NTT * E], FP32, tag="mx8")
                    t1g = gatep.tile([P, NTT], FP32, tag="t1g")
                    e1a = gatep.tile([P, NTT], FP32, tag="e1a")
                    e2a = gatep.tile([P, NTT], FP32, tag="e2a")
                    for t in range(NTT):
                        nc.vector.max(
                            out=mx8[:, t * E : (t + 1) * E],
                            in_=clsb[:, t * E : (t + 1) * E],
                        )
                        nc.vector.tensor_copy(
                            e1a[:, t : t + 1], mx8[:, t * E : t * E + 1]
                        )
                        nc.vector.tensor_copy(
                            e2a[:, t : t + 1], mx8[:, t * E + 1 : t * E + 2]
                        )
                    nc.vector.tensor_add(t1g[:], e1a[:], e2a[:])
                    nc.vector.reciprocal(t1g[:], t1g[:])
                    nc.vector.tensor_mul(g1sb[:], e1a[:], t1g[:])
                    # r = g2/g1 = e2/e1 for the fused combine (b1 + r*b2)
                    nc.vector.reciprocal(e1a[:], e1a[:])
                    nc.vector.tensor_mul(g2sb[:], e2a[:], e1a[:])

                # ---------- phase B: fc1 + gelu (fused pairs) ----------
                if "gate" in skip:
                    gfence = gatep.tile([P, 1], FP32, tag="gfence")
                    nc.vector.tensor_scalar_mul(gfence[:], g2sb[:, 0:1], 0.0)
                hall = []
                for k in range(E):
                    if k + 2 < E:
                        x_pre[k + 2] = load_x(k + 2)
                    if k + 1 < E:
                        w_pre[k + 1] = load_w(1, w1t_d, k + 1)
                    if k == E - 2:
                        w2_pre = {0: load_w(2, w2t_d, 0)}
                    if k == E - 1:
                        w2_pre[1] = load_w(2, w2t_d, 1)
                    w1sb, b1sb = w_pre.pop(k)
                    if use_b1:
                        b1f = wp.tile([P, HC], FP32, tag="b1f")
                        nc.vector.tensor_scalar_add(
                            b1f[:], b1sb[:], gfence[:, 0:1]
                        )
                    xk = x_pre.pop(k)
                    cap = caps[k]
                    hsb = hallp.tile([P, HC * cap], BF16, tag=f"h{k}")
                    for hp in range(HC2 if "fc1" not in skip else 1):
                        p2 = pp.tile([P, 2, 512], FP32, tag="fc1_ps", bufs=3)
                        for hh in range(2):
                            h = 2 * hp + hh
                            for d_ in range(DC):
                                nc.tensor.matmul(
                                    p2[:, hh, :cap],
                                    lhsT=w1sb[:, h * 512 + d_ * P : h * 512 + (d_ + 1) * P],
                                    rhs=xk[:, d_ * cap : (d_ + 1) * cap],
                                    start=(d_ == 0),
                                    stop=(d_ == DC - 1),
                                )
                        if "gelu" in skip:
                            nc.vector.tensor_copy(
                                hsb[:, 2 * hp * cap : (2 * hp + 2) * cap],
                                p2[:, :, :cap],
                            )
                        elif use_b1:
                            for hh in range(2):
                                h = 2 * hp + hh
                                nc.scalar.activation(
                                    hsb[:, h * cap : (h + 1) * cap],
                                    p2[:, hh, :cap],
                                    gelu_af,
                                    bias=b1f[:, h : h + 1],
                                )
                        else:
                            nc.scalar.activation(
                                hsb[:, 2 * hp * cap : (2 * hp + 2) * cap].rearrange(
                                    "p (two c) -> p two c", two=2
                                ),
                                p2[:, :, :cap],
                                gelu_af,
                                bias=gfence[:, 0:1],
                            )
                    hall.append(hsb)

                # ---------- phase C: fc2 + exp + scatter + combine ----------
                lastc = HC * caps[E - 1]
                fence = gatep.tile([P, 1], FP32, tag="fence")
                nc.vector.tensor_scalar_mul(
                    fence[:], hall[E - 1][:, lastc - 1 : lastc], 0.0
                )
                if dbg:
                    tab = nc.declare_dram_parameter(
                        "tabd", [TROWS, D], BF16, isOutput=True
                    )
                    gdump = nc.declare_dram_parameter(
                        "gdump", [P, 2 * NTT], FP32, isOutput=True
                    )
                    nc.sync.dma_start(out=gdump[:, :NTT], in_=g1sb[:])
                    nc.sync.dma_start(out=gdump[:, NTT:], in_=g2sb[:])
                    hdump = nc.declare_dram_parameter(
                        "hdump", [P, HC * caps[0]], BF16, isOutput=True
                    )
                    nc.sync.dma_start(out=hdump[:], in_=hall[0][:])
                else:
                    tab = dp.tile([TROWS, D], BF16, tag="a_tab")

                comb_reads = []
                q_latest = {}

                def emit_combine(t0, L):
                    # combine L (1-2) adjacent token tiles. The table is
                    # interleaved (row = tile*256 + half*128 + p) so ONE read
                    # fetches both experts' rows; the per-tile math is a
                    # single fused DVE op t = b2*r + b1 (r = g2/g1, stored in
                    # g2sb) and y = Ln(t * g1) via the activation scale.
                    bg = cp.tile([P, L * 2 * D], BF16, tag="bg", bufs=3)
                    snap = dict(q_latest)
                    r1 = nc.sync.dma_start(
                        out=bg[:].rearrange("p (l h j) -> p l h j", l=L, h=2),
                        in_=tab[t0 * 2 * P : (t0 + L) * 2 * P, :].rearrange(
                            "(l h p) j -> p l h j", l=L, h=2
                        ),
                    )
                    comb_reads.append((r1, snap))
                    sv = cps_.tile([P, L * D], FP32, tag="sv", bufs=2)
                    yv = cps_.tile([P, L * D], BF16, tag="yv", bufs=2)
                    for l in range(L):
                        t = t0 + l
                        nc.vector.scalar_tensor_tensor(
                            out=sv[:, l * D : (l + 1) * D],
                            in0=bg[:, (2 * l + 1) * D : (2 * l + 2) * D],
                            scalar=g2sb[:, t : t + 1],
                            in1=bg[:, 2 * l * D : (2 * l + 1) * D],
                            op0=mybir.AluOpType.mult,
                            op1=mybir.AluOpType.add,
                        )
                        nc.scalar.activation(
                            yv[:, l * D : (l + 1) * D],
                            sv[:, l * D : (l + 1) * D],
                            AF.Ln,
                            scale=g1sb[:, t : t + 1],
                        )
                    nc.scalar.dma_start(
                        out=y_d[t0 * P : (t0 + L) * P, :].rearrange(
                            "(l p) j -> p l j", l=L
                        ),
                        in_=yv[:].rearrange("p (l j) -> p l j", l=L),
                    )

                scatters = []
                ch = 0
                for k in range(E):
                    if k + 2 < E:
                        w2_pre[k + 2] = load_w(2, w2t_d, k + 2)
                    w2sb, b2sb = w2_pre.pop(k)
                    cap = caps[k]
                    hsb = hall[k]
                    for tt in range(ntts[k]):
                        m = min(P, cap - tt * P)
                        asb = ap_.tile([P, D], BF16, tag="a_sb")
                        ps2t = pp.tile([P, D], FP32, tag="fc2_ps", bufs=ps2)
                        nh = HC if "fc2" not in skip else 1
                        for h in range(nh):
                            nc.tensor.matmul(
                                ps2t[:m],
                                lhsT=hsb[:, h * cap + tt * P : h * cap + tt * P + m],
                                rhs=w2sb[:, h * D : (h + 1) * D],
                                start=(h == 0),
                                stop=(h == nh - 1 and not use_b2),
                            )
                        if use_b2:
                            nc.tensor.matmul(
                                ps2t[:m],
                                lhsT=ones1[:, :m],
                                rhs=b2sb[:],
                                start=False,
                                stop=True,
                            )
                        nc.scalar.activation(
                            asb[:m], ps2t[:m], AF.Exp, bias=fence[:m, 0:1]
                        )
                        if "scatter" not in skip:
                            mm = max(m, 2)
                            si = nc.gpsimd.indirect_dma_start(
                                out=tab[:],
                                out_offset=bass.IndirectOffsetOnAxis(
                                    ap=sidxsb[:mm, ch : ch + 1], axis=0
                                ),
                                in_=asb[:mm],
                                in_offset=None,
                            )
                            si.ins.queue = f"qPoolDynamic{(ch % NQ) or ''}"
                            q_latest[ch % NQ] = si
                            scatters.append(si)
                        ch += 1
                    if "tail" not in skip:
                        t = 0
                        while t < NTT:
                            if rsegs[t] == k:
                                L = 2 if (t + 1 < NTT and rsegs[t + 1] == k) else 1
                                emit_combine(t, L)
                                t += L
                            else:
                                t += 1
                # scatter destination rows are disjoint (injective dest map):
                # strip false WAW/WAR deps from the tracker's full-range AP so
                # scatters pipeline; combine reads then explicitly wait on the
                # latest scatter of every queue emitted before them.
                false_dep = {i.ins.name for i in scatters} | {
                    r.ins.name for r, _ in comb_reads
                }
                for si in scatters:
                    for nm in list(si.ins.sync_dependency_names()):
                        if nm in false_dep:
                            si.ins.try_remove_dependency(nm)
                dinfo = None
                for ri, snap in comb_reads:
                    have = set(ri.ins.sync_dependency_names())
                    if dinfo is None and have:
                        dinfo = ri.ins.get_dependency_info(next(iter(have)))
                    for si in snap.values():
                        if si.ins.name not in have:
                            ri.ins.add_dependency(si.ins.name, dinfo)

            if reps > 1:
                U = unroll
                while reps % U:
                    U -= 1
                with tc.For_i(0, reps // U, 1, staggered_reset=True):
                    for _u in range(U):
                        body()
            else:
                body()
            if timing:
                nc.sync.dma_start(out=yo_d[:], in_=dummy4[:])

    nc.compile()
    return nc


def _route(gate_feat, noise, w_gate, w_noise):
    """Host-side routing structure (fp32 numpy, matches jax top-k selection)."""
    clean = gate_feat @ w_gate
    stddev = np.logaddexp(gate_feat @ w_noise, 0.0) + np.float32(1e-2)
    logits = clean.astype(np.float32) + noise * stddev.astype(np.float32)
    top2 = np.argsort(-logits, axis=1, kind="stable")[:, :TOPK].astype(np.int32)
    return top2


def _prepare(x, gate_feat, noise, w_gate, w_noise, fc1_w, fc1_b, fc2_w, fc2_b):
    x = np.ascontiguousarray(x, dtype=np.float32)
    gate_feat = np.ascontiguousarray(gate_feat, dtype=np.float32)
    noise = np.ascontiguousarray(noise, dtype=np.float32)

    top2 = _route(gate_feat, noise, w_gate, w_noise)

    bf = ml_dtypes.bfloat16
    # w1l[e, p, h*512 + d*128 + q] = fc1_w[e, h*128+q, d*128+p]
    w1l_all = np.ascontiguousarray(
        fc1_w.reshape(E, HC, P, DC, P).transpose(0, 4, 1, 3, 2).reshape(E, P, DC * H)
    ).astype(bf)
    # w2l[e, p, h*512 + j] = fc2_w[e, j, h*128+p]
    w2l_all = np.ascontiguousarray(
        fc2_w.reshape(E, D, HC, P).transpose(0, 3, 2, 1).reshape(E, P, HC * D)
    ).astype(bf)
    b2_all = np.ascontiguousarray(fc2_b).astype(bf)
    wgwn = np.hstack([w_gate, w_noise]).astype(np.float32)
    # wgwn2[p, c*16+j] = wgwn[c*128+p, j]
    wgwn2 = np.ascontiguousarray(
        wgwn.reshape(FC, P, 2 * E).transpose(1, 0, 2).reshape(P, FC * 2 * E)
    ).astype(bf)

    core_meta = []
    for c in range(NC):
        t2 = top2[c * NS : (c + 1) * NS]
        cnt = np.bincount(t2.ravel(), minlength=E)
        order = np.argsort(-cnt, kind="stable").astype(np.int32)
        seg_of_expert = np.empty(E, dtype=np.int64)
        seg_of_expert[order] = np.arange(E)
        pair_seg = seg_of_expert[t2.ravel()]
        sort_idx = np.argsort(pair_seg, kind="stable")
        seg_counts = cnt[order]
        core_meta.append((t2, order, pair_seg, sort_idx, seg_counts))

    caps = np.max(np.stack([m[4] for m in core_meta]), axis=0)
    offs = np.concatenate([[0], np.cumsum(caps)]).astype(np.int64)
    R = int(offs[-1])
    ntts = [(int(c) + P - 1) // P for c in caps]
    NCH = sum(ntts)

    in_maps = []
    perms = []
    rsegs_cores = []
    for c in range(NC):
        t2, order, pair_seg, sort_idx, seg_counts = core_meta[c]
        pos_in_seg = np.arange(2 * NS) - np.concatenate([[0], np.cumsum(seg_counts)])[pair_seg[sort_idx]]
        rows_sorted = offs[pair_seg[sort_idx]] + pos_in_seg
        rows_of_pair = np.empty(2 * NS, dtype=np.int64)
        rows_of_pair[sort_idx] = rows_sorted

        ready = np.maximum(pair_seg[0::2], pair_seg[1::2])
        perm = np.argsort(ready, kind="stable")
        inv_perm = np.empty(NS, dtype=np.int64)
        inv_perm[perm] = np.arange(NS)
        rseg_core = ready[perm].reshape(NTT, P).max(axis=1)

        # interleaved table: row = tile*256 + half*128 + p
        dest = np.empty(R + P, dtype=np.int32)
        dest[:] = 2 * NS + (np.arange(R + P) % P)
        base = (inv_perm // P) * (2 * P) + (inv_perm % P)
        dest[rows_of_pair[0::2]] = base
        dest[rows_of_pair[1::2]] = base + P
        sidx = np.zeros((P, NCH), dtype=np.int32)
        chv = 0
        for k in range(E):
            for tt in range(ntts[k]):
                s = int(offs[k]) + tt * P
                sidx[:, chv] = dest[s : s + P]
                # rows past this segment's capacity would alias the NEXT
                # segment's dest entries — route them to the dump region
                # (scatters run concurrently, so aliasing is a race)
                m = min(P, int(caps[k]) - tt * P)
                if m < P:
                    sidx[m:, chv] = 2 * NS + np.arange(m, P)
                chv += 1

        tok_sorted = sort_idx // 2
        cols = np.zeros(R, dtype=np.int64)
        for k in range(E):
            s0 = int(np.concatenate([[0], np.cumsum(seg_counts)])[k])
            cnt_k = int(seg_counts[k])
            cols[offs[k] : offs[k] + cnt_k] = tok_sorted[s0 : s0 + cnt_k]
        x_loc = x[c * NS : (c + 1) * NS]
        # xl[p, DC*offs[k] + d*cap_k + t] = x_loc[cols_k[t], d*128+p]
        xl = np.zeros((P, DC * R), dtype=bf)
        for k in range(E):
            o0, cap_k = int(offs[k]), int(caps[k])
            seg = x_loc[cols[o0 : o0 + cap_k]]           # [cap, D]
            xl[:, DC * o0 : DC * (o0 + cap_k)] = (
                seg.reshape(cap_k, DC, P).transpose(2, 1, 0).reshape(P, DC * cap_k)
            )

        gf_loc = gate_feat[c * NS : (c + 1) * NS]
        # gft[p, t*1024 + c*128 + q] = gf_loc[perm[t*128+q], c*128+p]
        gfp = gf_loc[perm]                                # [NS, 2D]
        gft = np.ascontiguousarray(
            gfp.reshape(NTT, P, FC, P).transpose(3, 0, 2, 1).reshape(P, NTT * FC * P)
        ).astype(bf)

        ns_loc = noise[c * NS : (c + 1) * NS]
        nst = np.ascontiguousarray(
            ns_loc[perm].reshape(NTT, P, E).transpose(1, 0, 2).reshape(P, NTT * E)
        ).astype(np.float32)
        im = {
            "xl": np.ascontiguousarray(xl),
            "gft": gft,
            "nst": nst,
            "wgwn": wgwn2,
            "w1t": np.ascontiguousarray(w1l_all[order]),
            "w2t": np.ascontiguousarray(w2l_all[order]),
            "b2": np.ascontiguousarray(b2_all[order]),
            "sidx": sidx,
        }
        if np.any(fc1_b):
            im["b1"] = np.ascontiguousarray(
                fc1_b[order].reshape(E, HC, P).transpose(0, 2, 1)
            ).astype(np.float32)
        in_maps.append(im)
        perms.append(perm)
        rsegs_cores.append(rseg_core)

    rsegs = tuple(int(v) for v in np.max(np.stack(rsegs_cores), axis=0))
    return caps, rsegs, perms, in_maps


def kernel(x, gate_feat, noise, w_gate, w_noise, fc1_w, fc1_b, fc2_w, fc2_b,
           _reps=1):
    caps, rsegs, perms, in_maps = _prepare(
        x, gate_feat, noise, w_gate, w_noise, fc1_w, fc1_b, fc2_w, fc2_b
    )
    use_b2 = bool(np.any(np.asarray(fc2_b)))
    use_b1 = bool(np.any(np.asarray(fc1_b)))
    key = (tuple(int(v) for v in caps), rsegs, int(_reps), use_b2, use_b1)
    if key not in _nc_cache:
        _nc_cache[key] = _build_nc(caps, rsegs, reps=_reps, use_b2=use_b2,
                                   use_b1=use_b1)
    nc = _nc_cache[key]
    try:
        res = run_bass_kernel_spmd(nc, in_maps, core_ids=list(range(NC)))
    except Exception:
        res = run_bass_kernel_spmd(nc, in_maps, core_ids=list(range(NC)))
    y = np.empty((N, D), np.float32)
    for c in range(NC):
        y[c * NS : (c + 1) * NS][perms[c]] = res.results[c]["y"].astype(np.float32)
    return y


# revision 32
# speedup vs baseline: 1.0224x; 1.0069x over previous
"""MoE (noisy top-2-of-8 gating) Trainium2 kernel — v2.

Strategy: data-parallel over tokens (1024/core on 8 cores). Host computes
routing structure only; all FLOPs (gating values, expert MLPs, combine) run
on device. Tokens are permuted into 8 expert segments (experts sorted by
descending count; per-segment capacity = max count over cores so one SPMD
program serves all cores).

v2 changes vs the original (218994ns) kernel:
  * All DRAM operands are host-relaid so each logical load is 1-4 large
    DMAs (2KB+/partition lines): 8 gate-feature tile loads, 4 chunks per
    expert for fc1/fc2 weights, 1 per expert for x segments. HWDGE
    descriptor-generation drops from 218 ops (136us serialized) to ~90.
  * fc1 psums are built two h-chunks per 2-bank PSUM tile and drained by
    ONE gelu per pair: the ~390ns fixed ACT cost amortizes over 2x the
    elements, phase B stops being ACT-bound (78us -> ~53us of ACT).
  * The scatter table is interleaved (row = tile*256 + half*128 + p) so
    each combine group (1-2 token tiles sharing a readiness segment) is
    ONE strided read instead of 2-4; the combine math collapses to one
    fused DVE op per tile, t = b2*(g2/g1) + b1 via scalar_tensor_tensor,
    and y = Ln(t*g1) via the activation scale — the old mul/mul/add/Ln
    chain (4 ops, 2 round trips) becomes 2 ops.
  * Combine reads ride the SP HWDGE queue (measured faster than Pool,
    where their scatter-sem wait blocks later scatters' desc-gen) and y
    stores ride ACT; the Pool engine keeps only the 21 solo scatters.
    (Paired 2-column scatters compute wrong on HW — the DynamicAP index
    unroll order differs from CoreSim — so scatters stay one column per
    fc2 output tile. A 6-deep asb ring decouples scatter latency from
    the exp/PSUM drain.)
  * y is stored bf16 (host upcasts); adds ~2e-4 rel err, halves tail
    store time.

The Exp/Ln table-set chooser in bass is naive (picks the first set with
the function, thrashing between exp_and_others/natural_log on every
transition) — we point both at natural_log_exp_and_others (which
genuinely contains both) so each phase boundary is one table load.
"""

import numpy as np
import ml_dtypes

import concourse.bacc as bacc
import concourse.bass as bass
import concourse.mybir as mybir
import concourse.tile as tile
import concourse.hw_specs as hw_specs
from concourse.bass_utils import run_bass_kernel_spmd

BF16 = mybir.dt.bfloat16
FP32 = mybir.dt.float32
AF = mybir.ActivationFunctionType

N, D, H, E, TOPK = 8192, 512, 2048, 8, 2
NC = 8
NS = N // NC          # tokens per core
P = 128
NTT = NS // P         # token tiles per core (8)
DC = D // P           # d chunks (4)
HC = H // P           # hidden chunks (16)
HC2 = HC // 2         # fused gelu pairs (8)
FC = (2 * D) // P     # gate feature chunks (8)
NQ = 4                # SWDGE queues (hardware max)
WCH = 4               # weight load chunks per expert

_nc_cache: dict = {}
_act_tables_patched = [False]


def _patch_act_tables(arch: str):
    """Make Exp and Ln both resolve to natural_log_exp_and_others so the
    act-table fixpoint emits one load per phase instead of one per
    Exp<->Ln transition."""
    tabs = hw_specs.get_activation_tables(arch)
    if "natural_log_exp_and_others" in tabs:
        both = tabs["natural_log_exp_and_others"]
        if AF.Exp in both and AF.Ln in both:
            tabs["exp_and_others"].discard(AF.Exp)
            tabs["natural_log"].discard(AF.Ln)
    _act_tables_patched[0] = True


def _build_nc(caps, rsegs=(7,) * 8, reps=1, gelu_sub=False, timing=False,
              skip=(), wbufs=4, ps2=2, use_b2=True, use_b1=False, unroll=8,
              dbg=False, gsb_bufs=1):
    """Build the SPMD Bass program for per-segment capacities `caps`."""
    gelu_af = AF.Tanh if gelu_sub else AF.Gelu
    caps = tuple(int(c) for c in caps)
    R = sum(caps)
    offs = np.concatenate([[0], np.cumsum(caps)]).astype(int)
    ntts = [(c + P - 1) // P for c in caps]
    NCH = sum(ntts)              # fc2 output tiles == scatter chunk columns
    TROWS = 2 * NS + P           # interleaved token tables | dump rows

    nc = bacc.Bacc("TRN2", target_bir_lowering=False, debug=False,
                   num_swdge_queues=NQ)
    if not _act_tables_patched[0]:
        _patch_act_tables(nc.m.arch)

    if timing:
        def param(name, shape, dtype):
            return nc.dram_tensor(name, shape, dtype)
        dummy_d = nc.declare_dram_parameter("tdin", [1, 4], FP32, isOutput=False)
        y_d = nc.dram_tensor("y", [NS, D], BF16)
        yo_d = nc.declare_dram_parameter("yo", [1, 4], FP32, isOutput=True)
    else:
        def param(name, shape, dtype):
            return nc.declare_dram_parameter(name, shape, dtype, isOutput=False)
        y_d = nc.declare_dram_parameter("y", [NS, D], BF16, isOutput=True)

    xl_d = param("xl", [P, DC * R], BF16)
    gft_d = param("gft", [P, NTT * FC * P], BF16)
    nst_d = param("nst", [P, NTT * E], FP32)
    wgwn_d = param("wgwn", [P, FC * 2 * E], BF16)
    w1t_d = param("w1t", [E, P, DC * H], BF16)
    w2t_d = param("w2t", [E, P, HC * D], BF16)
    b2_d = param("b2", [E, D], BF16)
    b1_d = param("b1", [E, P, HC], FP32) if use_b1 else None
    sidx_d = param("sidx", [P, NCH], mybir.dt.int32)

    with tile.TileContext(nc) as tc:
        with (
            tc.tile_pool(name="const", bufs=1) as constp,
            tc.tile_pool(name="gate", bufs=1) as gatep,
            tc.tile_pool(name="hall", bufs=1) as hallp,
            tc.tile_pool(name="xpool", bufs=1) as xp,
            tc.tile_pool(name="wpool", bufs=wbufs) as wp,
            tc.tile_pool(name="apool", bufs=6) as ap_,
            tc.tile_pool(name="cpool", bufs=4) as cp,
            tc.tile_pool(name="cspool", bufs=3) as cps_,
            tc.tile_pool(name="psum", bufs=1, space="PSUM") as pp,
            tc.tile_pool(name="dram", bufs=1, space="DRAM") as dp,
        ):
            ones1 = constp.tile([1, P], BF16)
            nc.vector.memset(ones1[:], 1.0)
            dummy4 = constp.tile([1, 4], FP32)
            nc.vector.memset(dummy4[:], 0.0)

            def load_w(which, d_, k):
                wsb = wp.tile([P, DC * H], BF16, tag="w")
                step = (DC * H) // WCH
                for c in range(WCH if "wdma" not in skip else 1):
                    nc.sync.dma_start(
                        out=wsb[:, c * step : (c + 1) * step],
                        in_=d_[k, :, c * step : (c + 1) * step],
                    )
                if which == 2 and use_b2:
                    b2sb = wp.tile([1, D], BF16, tag="b2")
                    nc.sync.dma_start(out=b2sb[:], in_=b2_d[k][None, :])
                    return wsb, b2sb
                if which == 1 and use_b1:
                    b1sb = wp.tile([P, HC], FP32, tag="b1")
                    nc.sync.dma_start(out=b1sb[:], in_=b1_d[k])
                    return wsb, b1sb
                return wsb, None

            def load_x(k):
                cap = caps[k]
                o0 = int(offs[k])
                xk = xp.tile([P, DC * cap], BF16, tag=f"x{k}")
                nc.sync.dma_start(out=xk[:], in_=xl_d[:, DC * o0 : DC * (o0 + cap)])
                return xk

            def body(_i=None):
                # ---------- persistent loads (gating first) ----------
                gsb = gatep.tile([P, NTT * FC * P], BF16, tag="gsb",
                                 bufs=gsb_bufs)
                for t in range(NTT):
                    nc.sync.dma_start(
                        out=gsb[:, t * FC * P : (t + 1) * FC * P],
                        in_=gft_d[:, t * FC * P : (t + 1) * FC * P],
                    )
                wg2sb = gatep.tile([P, FC * 2 * E], BF16, tag="wg2sb")
                nc.sync.dma_start(out=wg2sb[:], in_=wgwn_d[:])
                nssb = gatep.tile([P, NTT * E], FP32, tag="nssb")
                nc.sync.dma_start(out=nssb[:], in_=nst_d[:])
                sidxsb = gatep.tile([P, NCH], mybir.dt.int32, tag="sidxsb")
                if timing:
                    nc.vector.memset(sidxsb[:], 0)
                else:
                    nc.sync.dma_start(out=sidxsb[:], in_=sidx_d[:])
                # x0 before the bulk of w1(0): the first fc1 psum pair needs
                # only w1 chunk 0 + x0, and x0 is the smaller transfer
                x_pre = {0: load_x(0)}
                w_pre = {0: load_w(1, w1t_d, 0)}
                x_pre[1] = load_x(1)

                # ---------- phase A: gating, token-major ----------
                g1sb = gatep.tile([P, NTT], FP32, tag="g1")
                g2sb = gatep.tile([P, NTT], FP32, tag="g2")
                if "gate" in skip:
                    nc.vector.memset(g1sb[:], 0.5)
                    nc.vector.memset(g2sb[:], 0.5)
                else:
                    W2E = 2 * E
                    clsb = gatep.tile([P, NTT * E], FP32, tag="clsb")
                    nssp = gatep.tile([P, NTT * E], FP32, tag="nssp")
                    for t in range(NTT):
                        gpsw = pp.tile([P, 2, 512], FP32, tag="fc1_ps", bufs=3)
                        gps = gpsw[:, 0, :W2E]
                        for c in range(FC):
                            nc.tensor.matmul(
                                gps[:],
                                lhsT=gsb[:, t * FC * P + c * P : t * FC * P + (c + 1) * P],
                                rhs=wg2sb[:, c * W2E : (c + 1) * W2E],
                                start=(c == 0),
                                stop=(c == FC - 1),
                            )
                        nc.vector.tensor_copy(
                            clsb[:, t * E : (t + 1) * E], gps[:, :E]
                        )
                        nc.vector.tensor_copy(
                            nssp[:, t * E : (t + 1) * E], gps[:, E:]
                        )
                    # stddev = softplus(noise logits) + 1e-2
                    nc.scalar.activation(nssp[:], nssp[:], AF.Exp)
                    nc.vector.tensor_scalar_add(nssp[:], nssp[:], 1.0)
                    nc.scalar.activation(nssp[:], nssp[:], AF.Ln)
                    nc.vector.tensor_scalar_add(nssp[:], nssp[:], 1e-2)
                    # logits = clean + noise * stddev; exp once — top-2 of
                    # exp(logits) == exp(top-2), so the gate softmax is pure
                    # DVE math after this
                    nc.vector.tensor_mul(nssp[:], nssp[:], nssb[:])
                    nc.vector.tensor_add(clsb[:], clsb[:], nssp[:])
                    nc.scalar.activation(clsb[:], clsb[:], AF.Exp)
                    # fence emitted BEFORE the g1/g2 math so gelus unblock as
                    # soon as the exp lands
                    gfence = gatep.tile([P, 1], FP32, tag="gfence")
                    nc.vector.tensor_scalar_mul(gfence[:], clsb[:, 0:1], 0.0)
                    mx8 = gatep.tile([P, NTT * E], FP32, tag="mx8")
                    t1g = gatep.tile([P, NTT], FP32, tag="t1g")
                    e1a = gatep.tile([P, NTT], FP32, tag="e1a")
                    e2a = gatep.tile([P, NTT], FP32, tag="e2a")
                    for t in range(NTT):
                        nc.vector.max(
                            out=mx8[:, t * E : (t + 1) * E],
                            in_=clsb[:, t * E : (t + 1) * E],
                        )
                        nc.vector.tensor_copy(
                            e1a[:, t : t + 1], mx8[:, t * E : t * E + 1]
                        )
                        nc.vector.tensor_copy(
                            e2a[:, t : t + 1], mx8[:, t * E + 1 : t * E + 2]
                        )
                    nc.vector.tensor_add(t1g[:], e1a[:], e2a[:])
                    nc.vector.reciprocal(t1g[:], t1g[:])
                    nc.vector.tensor_mul(g1sb[:], e1a[:], t1g[:])
                    # r = g2/g1 = e2/e1 for the fused combine (b1 + r*b2)
                    nc.vector.reciprocal(e1a[:], e1a[:])
                    nc.vector.tensor_mul(g2sb[:], e2a[:], e1a[:])

                # ---------- phase B: fc1 + gelu (fused pairs) ----------
                if "gate" in skip:
                    gfence = gatep.tile([P, 1], FP32, tag="gfence")
                    nc.vector.tensor_scalar_mul(gfence[:], g2sb[:, 0:1], 0.0)
                hall = []
                for k in range(E):
                    if k + 2 < E:
                        x_pre[k + 2] = load_x(k + 2)
                    if k + 1 < E:
                        w_pre[k + 1] = load_w(1, w1t_d, k + 1)
                    if k == E - 2:
                        w2_pre = {0: load_w(2, w2t_d, 0)}
                    if k == E - 1:
                        w2_pre[1] = load_w(2, w2t_d, 1)
                    w1sb, b1sb = w_pre.pop(k)
                    if use_b1:
                        b1f = wp.tile([P, HC], FP32, tag="b1f")
                        nc.vector.tensor_scalar_add(
                            b1f[:], b1sb[:], gfence[:, 0:1]
                        )
                    xk = x_pre.pop(k)
                    cap = caps[k]
                    hsb = hallp.tile([P, HC * cap], BF16, tag=f"h{k}")
                    for hp in range(HC2 if "fc1" not in skip else 1):
                        p2 = pp.tile([P, 2, 512], FP32, tag="fc1_ps", bufs=3)
                        for hh in range(2):
                            h = 2 * hp + hh
                            for d_ in range(DC):
                                nc.tensor.matmul(
                                    p2[:, hh, :cap],
                                    lhsT=w1sb[:, h * 512 + d_ * P : h * 512 + (d_ + 1) * P],
                                    rhs=xk[:, d_ * cap : (d_ + 1) * cap],
                                    start=(d_ == 0),
                                    stop=(d_ == DC - 1),
                                )
                        if "gelu" in skip:
                            nc.vector.tensor_copy(
                                hsb[:, 2 * hp * cap : (2 * hp + 2) * cap],
                                p2[:, :, :cap],
                            )
                        elif use_b1:
                            for hh in range(2):
                                h = 2 * hp + hh
                                nc.scalar.activation(
                                    hsb[:, h * cap : (h + 1) * cap],
                                    p2[:, hh, :cap],
                                    gelu_af,
                                    bias=b1f[:, h : h + 1],
                                )
                        else:
                            nc.scalar.activation(
                                hsb[:, 2 * hp * cap : (2 * hp + 2) * cap].rearrange(
                                    "p (two c) -> p two c", two=2
                                ),
                                p2[:, :, :cap],
                                gelu_af,
                                bias=gfence[:, 0:1],
                            )
                    hall.append(hsb)

                # ---------- phase C: fc2 + exp + scatter + combine ----------
                lastc = HC * caps[E - 1]
                fence = gatep.tile([P, 1], FP32, tag="fence")
                nc.vector.tensor_scalar_mul(
                    fence[:], hall[E - 1][:, lastc - 1 : lastc], 0.0
                )
                if dbg:
                    tab = nc.declare_dram_parameter(
                        "tabd", [TROWS, D], BF16, isOutput=True
                    )
                    gdump = nc.declare_dram_parameter(
                        "gdump", [P, 2 * NTT], FP32, isOutput=True
                    )
                    nc.sync.dma_start(out=gdump[:, :NTT], in_=g1sb[:])
                    nc.sync.dma_start(out=gdump[:, NTT:], in_=g2sb[:])
                    hdump = nc.declare_dram_parameter(
                        "hdump", [P, HC * caps[0]], BF16, isOutput=True
                    )
                    nc.sync.dma_start(out=hdump[:], in_=hall[0][:])
                else:
                    tab = dp.tile([TROWS, D], BF16, tag="a_tab")

                comb_reads = []
                q_latest = {}

                def emit_combine(t0, L):
                    # combine L (1-2) adjacent token tiles. The table is
                    # interleaved (row = tile*256 + half*128 + p) so ONE read
                    # fetches both experts' rows; the per-tile math is a
                    # single fused DVE op t = b2*r + b1 (r = g2/g1, stored in
                    # g2sb) and y = Ln(t * g1) via the activation scale.
                    bg = cp.tile([P, L * 2 * D], BF16, tag="bg", bufs=3)
                    snap = dict(q_latest)
                    # read on SP: its scatter-sem wait delays only the next
                    # iteration's input loads (slack there); on Pool it would
                    # block later scatters' desc-gen (measured 19us worse)
                    r1 = nc.sync.dma_start(
                        out=bg[:].rearrange("p (l h j) -> p l h j", l=L, h=2),
                        in_=tab[t0 * 2 * P : (t0 + L) * 2 * P, :].rearrange(
                            "(l h p) j -> p l h j", l=L, h=2
                        ),
                    )
                    comb_reads.append((r1, snap))
                    sv = cps_.tile([P, L * D], FP32, tag="sv", bufs=2)
                    yv = cps_.tile([P, L * D], BF16, tag="yv", bufs=2)
                    for l in range(L):
                        t = t0 + l
                        nc.vector.scalar_tensor_tensor(
                            out=sv[:, l * D : (l + 1) * D],
                            in0=bg[:, (2 * l + 1) * D : (2 * l + 2) * D],
                            scalar=g2sb[:, t : t + 1],
                            in1=bg[:, 2 * l * D : (2 * l + 1) * D],
                            op0=mybir.AluOpType.mult,
                            op1=mybir.AluOpType.add,
                        )
                        nc.scalar.activation(
                            yv[:, l * D : (l + 1) * D],
                            sv[:, l * D : (l + 1) * D],
                            AF.Ln,
                            scale=g1sb[:, t : t + 1],
                        )
                    nc.scalar.dma_start(
                        out=y_d[t0 * P : (t0 + L) * P, :].rearrange(
                            "(l p) j -> p l j", l=L
                        ),
                        in_=yv[:].rearrange("p (l j) -> p l j", l=L),
                    )

                scatters = []
                ch = 0
                for k in range(E):
                    if k + 2 < E:
                        w2_pre[k + 2] = load_w(2, w2t_d, k + 2)
                    w2sb, b2sb = w2_pre.pop(k)
                    cap = caps[k]
                    hsb = hall[k]
                    for tt in range(ntts[k]):
                        m = min(P, cap - tt * P)
                        asb = ap_.tile([P, D], BF16, tag="a_sb")
                        ps2t = pp.tile([P, D], FP32, tag="fc2_ps", bufs=ps2)
                        nh = HC if "fc2" not in skip else 1
                        for h in range(nh):
                            nc.tensor.matmul(
                                ps2t[:m],
                                lhsT=hsb[:, h * cap + tt * P : h * cap + tt * P + m],
                                rhs=w2sb[:, h * D : (h + 1) * D],
                                start=(h == 0),
                                stop=(h == nh - 1 and not use_b2),
                            )
                        if use_b2:
                            nc.tensor.matmul(
                                ps2t[:m],
                                lhsT=ones1[:, :m],
                                rhs=b2sb[:],
                                start=False,
                                stop=True,
                            )
                        nc.scalar.activation(
                            asb[:m], ps2t[:m], AF.Exp, bias=fence[:m, 0:1]
                        )
                        if "scatter" not in skip:
                            mm = max(m, 2)
                            si = nc.gpsimd.indirect_dma_start(
                                out=tab[:],
                                out_offset=bass.IndirectOffsetOnAxis(
                                    ap=sidxsb[:mm, ch : ch + 1], axis=0
                                ),
                                in_=asb[:mm],
                                in_offset=None,
                            )
                            si.ins.queue = f"qPoolDynamic{(ch % NQ) or ''}"
                            q_latest[ch % NQ] = si
                            scatters.append(si)
                        ch += 1
                    if "tail" not in skip:
                        t = 0
                        while t < NTT:
                            if rsegs[t] == k:
                                L = 2 if (t + 1 < NTT and rsegs[t + 1] == k) else 1
                                emit_combine(t, L)
                                t += L
                            else:
                                t += 1
                # scatter destination rows are disjoint (injective dest map):
                # strip false WAW/WAR deps from the tracker's full-range AP so
                # scatters pipeline; combine reads then explicitly wait on the
                # latest scatter of every queue emitted before them.
                false_dep = {i.ins.name for i in scatters} | {
                    r.ins.name for r, _ in comb_reads
                }
                for si in scatters:
                    for nm in list(si.ins.sync_dependency_names()):
                        if nm in false_dep:
                            si.ins.try_remove_dependency(nm)
                dinfo = None
                for ri, snap in comb_reads:
                    have = set(ri.ins.sync_dependency_names())
                    if dinfo is None and have:
                        dinfo = ri.ins.get_dependency_info(next(iter(have)))
                    for si in snap.values():
                        if si.ins.name not in have:
                            ri.ins.add_dependency(si.ins.name, dinfo)

            if reps > 1:
                U = unroll
                while reps % U:
                    U -= 1
                with tc.For_i(0, reps // U, 1, staggered_reset=True):
                    for _u in range(U):
                        body()
            else:
                body()
            if timing:
                nc.sync.dma_start(out=yo_d[:], in_=dummy4[:])

    nc.compile()
    return nc


def _route(gate_feat, noise, w_gate, w_noise):
    """Host-side routing structure (fp32 numpy, matches jax top-k selection)."""
    clean = gate_feat @ w_gate
    stddev = np.logaddexp(gate_feat @ w_noise, 0.0) + np.float32(1e-2)
    logits = clean.astype(np.float32) + noise * stddev.astype(np.float32)
    top2 = np.argsort(-logits, axis=1, kind="stable")[:, :TOPK].astype(np.int32)
    return top2


def _prepare(x, gate_feat, noise, w_gate, w_noise, fc1_w, fc1_b, fc2_w, fc2_b):
    x = np.ascontiguousarray(x, dtype=np.float32)
    gate_feat = np.ascontiguousarray(gate_feat, dtype=np.float32)
    noise = np.ascontiguousarray(noise, dtype=np.float32)

    top2 = _route(gate_feat, noise, w_gate, w_noise)

    bf = ml_dtypes.bfloat16
    # w1l[e, p, h*512 + d*128 + q] = fc1_w[e, h*128+q, d*128+p]
    w1l_all = np.ascontiguousarray(
        fc1_w.reshape(E, HC, P, DC, P).transpose(0, 4, 1, 3, 2).reshape(E, P, DC * H)
    ).astype(bf)
    # w2l[e, p, h*512 + j] = fc2_w[e, j, h*128+p]
    w2l_all = np.ascontiguousarray(
        fc2_w.reshape(E, D, HC, P).transpose(0, 3, 2, 1).reshape(E, P, HC * D)
    ).astype(bf)
    b2_all = np.ascontiguousarray(fc2_b).astype(bf)
    wgwn = np.hstack([w_gate, w_noise]).astype(np.float32)
    # wgwn2[p, c*16+j] = wgwn[c*128+p, j]
    wgwn2 = np.ascontiguousarray(
        wgwn.reshape(FC, P, 2 * E).transpose(1, 0, 2).reshape(P, FC * 2 * E)
    ).astype(bf)

    core_meta = []
    for c in range(NC):
        t2 = top2[c * NS : (c + 1) * NS]
        cnt = np.bincount(t2.ravel(), minlength=E)
        order = np.argsort(-cnt, kind="stable").astype(np.int32)
        seg_of_expert = np.empty(E, dtype=np.int64)
        seg_of_expert[order] = np.arange(E)
        pair_seg = seg_of_expert[t2.ravel()]
        sort_idx = np.argsort(pair_seg, kind="stable")
        seg_counts = cnt[order]
        core_meta.append((t2, order, pair_seg, sort_idx, seg_counts))

    caps = np.max(np.stack([m[4] for m in core_meta]), axis=0)
    offs = np.concatenate([[0], np.cumsum(caps)]).astype(np.int64)
    R = int(offs[-1])
    ntts = [(int(c) + P - 1) // P for c in caps]
    NCH = sum(ntts)

    in_maps = []
    perms = []
    rsegs_cores = []
    for c in range(NC):
        t2, order, pair_seg, sort_idx, seg_counts = core_meta[c]
        pos_in_seg = np.arange(2 * NS) - np.concatenate([[0], np.cumsum(seg_counts)])[pair_seg[sort_idx]]
        rows_sorted = offs[pair_seg[sort_idx]] + pos_in_seg
        rows_of_pair = np.empty(2 * NS, dtype=np.int64)
        rows_of_pair[sort_idx] = rows_sorted

        ready = np.maximum(pair_seg[0::2], pair_seg[1::2])
        perm = np.argsort(ready, kind="stable")
        inv_perm = np.empty(NS, dtype=np.int64)
        inv_perm[perm] = np.arange(NS)
        rseg_core = ready[perm].reshape(NTT, P).max(axis=1)

        # interleaved table: row = tile*256 + half*128 + p
        dest = np.empty(R + P, dtype=np.int32)
        dest[:] = 2 * NS + (np.arange(R + P) % P)
        base = (inv_perm // P) * (2 * P) + (inv_perm % P)
        dest[rows_of_pair[0::2]] = base
        dest[rows_of_pair[1::2]] = base + P
        sidx = np.zeros((P, NCH), dtype=np.int32)
        chv = 0
        for k in range(E):
            for tt in range(ntts[k]):
                s = int(offs[k]) + tt * P
                sidx[:, chv] = dest[s : s + P]
                # rows past this segment's capacity would alias the NEXT
                # segment's dest entries — route them to the dump region
                # (scatters run concurrently, so aliasing is a race)
                m = min(P, int(caps[k]) - tt * P)
                if m < P:
                    sidx[m:, chv] = 2 * NS + np.arange(m, P)
                chv += 1

        tok_sorted = sort_idx // 2
        cols = np.zeros(R, dtype=np.int64)
        for k in range(E):
            s0 = int(np.concatenate([[0], np.cumsum(seg_counts)])[k])
            cnt_k = int(seg_counts[k])
            cols[offs[k] : offs[k] + cnt_k] = tok_sorted[s0 : s0 + cnt_k]
        x_loc = x[c * NS : (c + 1) * NS]
        # xl[p, DC*offs[k] + d*cap_k + t] = x_loc[cols_k[t], d*128+p]
        xl = np.zeros((P, DC * R), dtype=bf)
        for k in range(E):
            o0, cap_k = int(offs[k]), int(caps[k])
            seg = x_loc[cols[o0 : o0 + cap_k]]           # [cap, D]
            xl[:, DC * o0 : DC * (o0 + cap_k)] = (
                seg.reshape(cap_k, DC, P).transpose(2, 1, 0).reshape(P, DC * cap_k)
            )

        gf_loc = gate_feat[c * NS : (c + 1) * NS]
        # gft[p, t*1024 + c*128 + q] = gf_loc[perm[t*128+q], c*128+p]
        gfp = gf_loc[perm]                                # [NS, 2D]
        gft = np.ascontiguousarray(
            gfp.reshape(NTT, P, FC, P).transpose(3, 0, 2, 1).reshape(P, NTT * FC * P)
        ).astype(bf)

        ns_loc = noise[c * NS : (c + 1) * NS]
        nst = np.ascontiguousarray(
            ns_loc[perm].reshape(NTT, P, E).transpose(1, 0, 2).reshape(P, NTT * E)
        ).astype(np.float32)
        im = {
            "xl": np.ascontiguousarray(xl),
            "gft": gft,
            "nst": nst,
            "wgwn": wgwn2,
            "w1t": np.ascontiguousarray(w1l_all[order]),
            "w2t": np.ascontiguousarray(w2l_all[order]),
            "b2": np.ascontiguousarray(b2_all[order]),
            "sidx": sidx,
        }
        if np.any(fc1_b):
            im["b1"] = np.ascontiguousarray(
                fc1_b[order].reshape(E, HC, P).transpose(0, 2, 1)
            ).astype(np.float32)
        in_maps.append(im)
        perms.append(perm)
        rsegs_cores.append(rseg_core)

    rsegs = tuple(int(v) for v in np.max(np.stack(rsegs_cores), axis=0))
    return caps, rsegs, perms, in_maps


def kernel(x, gate_feat, noise, w_gate, w_noise, fc1_w, fc1_b, fc2_w, fc2_b,
           _reps=1):
    caps, rsegs, perms, in_maps = _prepare(
        x, gate_feat, noise, w_gate, w_noise, fc1_w, fc1_b, fc2_w, fc2_b
    )
    use_b2 = bool(np.any(np.asarray(fc2_b)))
    use_b1 = bool(np.any(np.asarray(fc1_b)))
    key = (tuple(int(v) for v in caps), rsegs, int(_reps), use_b2, use_b1)
    if key not in _nc_cache:
        _nc_cache[key] = _build_nc(caps, rsegs, reps=_reps, use_b2=use_b2,
                                   use_b1=use_b1)
    nc = _nc_cache[key]
    try:
        res = run_bass_kernel_spmd(nc, in_maps, core_ids=list(range(NC)))
    except Exception:
        res = run_bass_kernel_spmd(nc, in_maps, core_ids=list(range(NC)))
    y = np.empty((N, D), np.float32)
    for c in range(NC):
        y[c * NS : (c + 1) * NS][perms[c]] = res.results[c]["y"].astype(np.float32)
    return y


# revision 34
# speedup vs baseline: 1.0313x; 1.0087x over previous
"""MoE (noisy top-2-of-8 gating) Trainium2 kernel — v2.

Strategy: data-parallel over tokens (1024/core on 8 cores). Host computes
routing structure only; all FLOPs (gating values, expert MLPs, combine) run
on device. Tokens are permuted into 8 expert segments (experts sorted by
descending count; per-segment capacity = max count over cores so one SPMD
program serves all cores).

v2 changes vs the original (218994ns) kernel:
  * All DRAM operands are host-relaid so each logical load is 1-4 large
    DMAs (2KB+/partition lines): 8 gate-feature tile loads, 4 chunks per
    expert for fc1/fc2 weights, 1 per expert for x segments. HWDGE
    descriptor-generation drops from 218 ops (136us serialized) to ~90.
  * fc1 psums are built two h-chunks per 2-bank PSUM tile and drained by
    ONE gelu per pair: the ~390ns fixed ACT cost amortizes over 2x the
    elements, phase B stops being ACT-bound (78us -> ~53us of ACT).
  * The scatter table is interleaved (row = tile*256 + half*128 + p) so
    each combine group (1-2 token tiles sharing a readiness segment) is
    ONE strided read instead of 2-4; the combine math collapses to one
    fused DVE op per tile, t = b2*(g2/g1) + b1 via scalar_tensor_tensor,
    and y = Ln(t*g1) via the activation scale — the old mul/mul/add/Ln
    chain (4 ops, 2 round trips) becomes 2 ops.
  * Combine reads ride the SP HWDGE queue (measured faster than Pool,
    where their scatter-sem wait blocks later scatters' desc-gen) and y
    stores ride ACT; the Pool engine keeps only the 21 solo scatters.
    (Paired 2-column scatters compute wrong on HW — the DynamicAP index
    unroll order differs from CoreSim — so scatters stay one column per
    fc2 output tile. A 6-deep asb ring decouples scatter latency from
    the exp/PSUM drain.)
  * y is stored bf16 (host upcasts); adds ~2e-4 rel err, halves tail
    store time.

The Exp/Ln table-set chooser in bass is naive (picks the first set with
the function, thrashing between exp_and_others/natural_log on every
transition) — we point both at natural_log_exp_and_others (which
genuinely contains both) so each phase boundary is one table load.
"""

import numpy as np
import ml_dtypes

import concourse.bacc as bacc
import concourse.bass as bass
import concourse.mybir as mybir
import concourse.tile as tile
import concourse.hw_specs as hw_specs
from concourse.bass_utils import run_bass_kernel_spmd

BF16 = mybir.dt.bfloat16
FP32 = mybir.dt.float32
AF = mybir.ActivationFunctionType

N, D, H, E, TOPK = 8192, 512, 2048, 8, 2
NC = 8
NS = N // NC          # tokens per core
P = 128
NTT = NS // P         # token tiles per core (8)
DC = D // P           # d chunks (4)
HC = H // P           # hidden chunks (16)
HC2 = HC // 2         # fused gelu pairs (8)
FC = (2 * D) // P     # gate feature chunks (8)
NQ = 4                # SWDGE queues (hardware max)
WCH = 4               # weight load chunks per expert

_nc_cache: dict = {}
_act_tables_patched = [False]


def _patch_act_tables(arch: str):
    """Make Exp and Ln both resolve to natural_log_exp_and_others so the
    act-table fixpoint emits one load per phase instead of one per
    Exp<->Ln transition."""
    tabs = hw_specs.get_activation_tables(arch)
    if "natural_log_exp_and_others" in tabs:
        both = tabs["natural_log_exp_and_others"]
        if AF.Exp in both and AF.Ln in both:
            tabs["exp_and_others"].discard(AF.Exp)
            tabs["natural_log"].discard(AF.Ln)
    _act_tables_patched[0] = True


def _build_nc(caps, rsegs=(7,) * 8, reps=1, gelu_sub=False, timing=False,
              skip=(), wbufs=4, ps2=2, use_b2=True, use_b1=False, unroll=8,
              dbg=False, gsb_bufs=1):
    """Build the SPMD Bass program for per-segment capacities `caps`."""
    gelu_af = AF.Tanh if gelu_sub else AF.Gelu
    caps = tuple(int(c) for c in caps)
    R = sum(caps)
    offs = np.concatenate([[0], np.cumsum(caps)]).astype(int)
    ntts = [(c + P - 1) // P for c in caps]
    NCH = sum(ntts)              # fc2 output tiles == scatter chunk columns
    TROWS = 2 * NS + P           # interleaved token tables | dump rows

    nc = bacc.Bacc("TRN2", target_bir_lowering=False, debug=False,
                   num_swdge_queues=NQ)
    if not _act_tables_patched[0]:
        _patch_act_tables(nc.m.arch)

    if timing:
        def param(name, shape, dtype):
            return nc.dram_tensor(name, shape, dtype)
        dummy_d = nc.declare_dram_parameter("tdin", [1, 4], FP32, isOutput=False)
        y_d = nc.dram_tensor("y", [NS, D], BF16)
        yo_d = nc.declare_dram_parameter("yo", [1, 4], FP32, isOutput=True)
    else:
        def param(name, shape, dtype):
            return nc.declare_dram_parameter(name, shape, dtype, isOutput=False)
        y_d = nc.declare_dram_parameter("y", [NS, D], BF16, isOutput=True)

    xl_d = param("xl", [P, DC * R], BF16)
    gft_d = param("gft", [P, NTT * FC * P], BF16)
    nst_d = param("nst", [P, NTT * E], FP32)
    wgwn_d = param("wgwn", [P, FC * 2 * E], BF16)
    w1t_d = param("w1t", [E, P, DC * H], BF16)
    w2t_d = param("w2t", [E, P, HC * D], BF16)
    b2_d = param("b2", [E, D], BF16)
    b1_d = param("b1", [E, P, HC], FP32) if use_b1 else None
    sidx_d = param("sidx", [P, NCH], mybir.dt.int32)

    with tile.TileContext(nc) as tc:
        with (
            tc.tile_pool(name="const", bufs=1) as constp,
            tc.tile_pool(name="gate", bufs=1) as gatep,
            tc.tile_pool(name="hall", bufs=1) as hallp,
            tc.tile_pool(name="xpool", bufs=1) as xp,
            tc.tile_pool(name="wpool", bufs=wbufs) as wp,
            tc.tile_pool(name="apool", bufs=6) as ap_,
            tc.tile_pool(name="cpool", bufs=4) as cp,
            tc.tile_pool(name="cspool", bufs=3) as cps_,
            tc.tile_pool(name="psum", bufs=1, space="PSUM") as pp,
            tc.tile_pool(name="dram", bufs=1, space="DRAM") as dp,
        ):
            ones1 = constp.tile([1, P], BF16)
            nc.vector.memset(ones1[:], 1.0)
            dummy4 = constp.tile([1, 4], FP32)
            nc.vector.memset(dummy4[:], 0.0)

            def load_w(which, d_, k):
                wsb = wp.tile([P, DC * H], BF16, tag="w")
                step = (DC * H) // WCH
                for c in range(WCH if "wdma" not in skip else 1):
                    nc.sync.dma_start(
                        out=wsb[:, c * step : (c + 1) * step],
                        in_=d_[k, :, c * step : (c + 1) * step],
                    )
                if which == 2 and use_b2:
                    b2sb = wp.tile([1, D], BF16, tag="b2")
                    nc.sync.dma_start(out=b2sb[:], in_=b2_d[k][None, :])
                    return wsb, b2sb
                if which == 1 and use_b1:
                    b1sb = wp.tile([P, HC], FP32, tag="b1")
                    nc.sync.dma_start(out=b1sb[:], in_=b1_d[k])
                    return wsb, b1sb
                return wsb, None

            def load_x(k):
                cap = caps[k]
                o0 = int(offs[k])
                xk = xp.tile([P, DC * cap], BF16, tag=f"x{k}")
                nc.sync.dma_start(out=xk[:], in_=xl_d[:, DC * o0 : DC * (o0 + cap)])
                return xk

            def body(_i=None):
                # ---------- persistent loads (gating first) ----------
                gsb = gatep.tile([P, NTT * FC * P], BF16, tag="gsb",
                                 bufs=gsb_bufs)
                for t in range(NTT):
                    nc.sync.dma_start(
                        out=gsb[:, t * FC * P : (t + 1) * FC * P],
                        in_=gft_d[:, t * FC * P : (t + 1) * FC * P],
                    )
                wg2sb = gatep.tile([P, FC * 2 * E], BF16, tag="wg2sb")
                nc.sync.dma_start(out=wg2sb[:], in_=wgwn_d[:])
                nssb = gatep.tile([P, NTT * E], FP32, tag="nssb")
                nc.sync.dma_start(out=nssb[:], in_=nst_d[:])
                sidxsb = gatep.tile([P, NCH], mybir.dt.int32, tag="sidxsb")
                if timing:
                    nc.vector.memset(sidxsb[:], 0)
                else:
                    nc.sync.dma_start(out=sidxsb[:], in_=sidx_d[:])
                # x0 before the bulk of w1(0): the first fc1 psum pair needs
                # only w1 chunk 0 + x0, and x0 is the smaller transfer
                x_pre = {0: load_x(0)}
                w_pre = {0: load_w(1, w1t_d, 0)}
                x_pre[1] = load_x(1)

                # ---------- phase A: gating, token-major ----------
                g1sb = gatep.tile([P, NTT], FP32, tag="g1")
                g2sb = gatep.tile([P, NTT], FP32, tag="g2")
                if "gate" in skip:
                    nc.vector.memset(g1sb[:], 0.5)
                    nc.vector.memset(g2sb[:], 0.5)
                else:
                    W2E = 2 * E
                    clsb = gatep.tile([P, NTT * E], FP32, tag="clsb")
                    nssp = gatep.tile([P, NTT * E], FP32, tag="nssp")
                    for t in range(NTT):
                        gpsw = pp.tile([P, 2, 512], FP32, tag="fc1_ps", bufs=3)
                        gps = gpsw[:, 0, :W2E]
                        for c in range(FC):
                            nc.tensor.matmul(
                                gps[:],
                                lhsT=gsb[:, t * FC * P + c * P : t * FC * P + (c + 1) * P],
                                rhs=wg2sb[:, c * W2E : (c + 1) * W2E],
                                start=(c == 0),
                                stop=(c == FC - 1),
                            )
                        nc.vector.tensor_copy(
                            clsb[:, t * E : (t + 1) * E], gps[:, :E]
                        )
                        nc.vector.tensor_copy(
                            nssp[:, t * E : (t + 1) * E], gps[:, E:]
                        )
                    # stddev = softplus(noise logits) + 1e-2
                    nc.scalar.activation(nssp[:], nssp[:], AF.Exp)
                    nc.vector.tensor_scalar_add(nssp[:], nssp[:], 1.0)
                    nc.scalar.activation(nssp[:], nssp[:], AF.Ln)
                    nc.vector.tensor_scalar_add(nssp[:], nssp[:], 1e-2)
                    # logits = clean + noise * stddev; exp once — top-2 of
                    # exp(logits) == exp(top-2), so the gate softmax is pure
                    # DVE math after this
                    nc.vector.tensor_mul(nssp[:], nssp[:], nssb[:])
                    nc.vector.tensor_add(clsb[:], clsb[:], nssp[:])
                    nc.scalar.activation(clsb[:], clsb[:], AF.Exp)
                    # fence emitted BEFORE the g1/g2 math so gelus unblock as
                    # soon as the exp lands
                    gfence = gatep.tile([P, 1], FP32, tag="gfence")
                    nc.vector.tensor_scalar_mul(gfence[:], clsb[:, 0:1], 0.0)
                    mx8 = gatep.tile([P, NTT * E], FP32, tag="mx8")
                    t1g = gatep.tile([P, NTT], FP32, tag="t1g")
                    e1a = gatep.tile([P, NTT], FP32, tag="e1a")
                    e2a = gatep.tile([P, NTT], FP32, tag="e2a")
                    for t in range(NTT):
                        nc.vector.max(
                            out=mx8[:, t * E : (t + 1) * E],
                            in_=clsb[:, t * E : (t + 1) * E],
                        )
                        nc.vector.tensor_copy(
                            e1a[:, t : t + 1], mx8[:, t * E : t * E + 1]
                        )
                        nc.vector.tensor_copy(
                            e2a[:, t : t + 1], mx8[:, t * E + 1 : t * E + 2]
                        )
                    nc.vector.tensor_add(t1g[:], e1a[:], e2a[:])
                    nc.vector.reciprocal(t1g[:], t1g[:])
                    nc.vector.tensor_mul(g1sb[:], e1a[:], t1g[:])
                    # r = g2/g1 = e2/e1 for the fused combine (b1 + r*b2)
                    nc.vector.reciprocal(e1a[:], e1a[:])
                    nc.vector.tensor_mul(g2sb[:], e2a[:], e1a[:])

                # ---------- phase B: fc1 + gelu (fused pairs) ----------
                if "gate" in skip:
                    gfence = gatep.tile([P, 1], FP32, tag="gfence")
                    nc.vector.tensor_scalar_mul(gfence[:], g2sb[:, 0:1], 0.0)
                hall = []
                for k in range(E):
                    if k + 2 < E:
                        x_pre[k + 2] = load_x(k + 2)
                    if k + 1 < E:
                        w_pre[k + 1] = load_w(1, w1t_d, k + 1)
                    if k == E - 2:
                        w2_pre = {0: load_w(2, w2t_d, 0)}
                    if k == E - 1:
                        w2_pre[1] = load_w(2, w2t_d, 1)
                    w1sb, b1sb = w_pre.pop(k)
                    if use_b1:
                        b1f = wp.tile([P, HC], FP32, tag="b1f")
                        nc.vector.tensor_scalar_add(
                            b1f[:], b1sb[:], gfence[:, 0:1]
                        )
                    xk = x_pre.pop(k)
                    cap = caps[k]
                    hsb = hallp.tile([P, HC * cap], BF16, tag=f"h{k}")
                    for hp in range(HC2 if "fc1" not in skip else 1):
                        p2 = pp.tile([P, 2, 512], FP32, tag="fc1_ps", bufs=3)
                        for hh in range(2):
                            h = 2 * hp + hh
                            for d_ in range(DC):
                                nc.tensor.matmul(
                                    p2[:, hh, :cap],
                                    lhsT=w1sb[:, h * 512 + d_ * P : h * 512 + (d_ + 1) * P],
                                    rhs=xk[:, d_ * cap : (d_ + 1) * cap],
                                    start=(d_ == 0),
                                    stop=(d_ == DC - 1),
                                )
                        if "gelu" in skip:
                            nc.vector.tensor_copy(
                                hsb[:, 2 * hp * cap : (2 * hp + 2) * cap],
                                p2[:, :, :cap],
                            )
                        elif use_b1:
                            for hh in range(2):
                                h = 2 * hp + hh
                                nc.scalar.activation(
                                    hsb[:, h * cap : (h + 1) * cap],
                                    p2[:, hh, :cap],
                                    gelu_af,
                                    bias=b1f[:, h : h + 1],
                                )
                        else:
                            nc.scalar.activation(
                                hsb[:, 2 * hp * cap : (2 * hp + 2) * cap].rearrange(
                                    "p (two c) -> p two c", two=2
                                ),
                                p2[:, :, :cap],
                                gelu_af,
                                bias=gfence[:, 0:1],
                            )
                    hall.append(hsb)

                # ---------- phase C: fc2 + exp + scatter + combine ----------
                lastc = HC * caps[E - 1]
                fence = gatep.tile([P, 1], FP32, tag="fence")
                nc.vector.tensor_scalar_mul(
                    fence[:], hall[E - 1][:, lastc - 1 : lastc], 0.0
                )
                if dbg:
                    tab = nc.declare_dram_parameter(
                        "tabd", [TROWS, D], BF16, isOutput=True
                    )
                    gdump = nc.declare_dram_parameter(
                        "gdump", [P, 2 * NTT], FP32, isOutput=True
                    )
                    nc.sync.dma_start(out=gdump[:, :NTT], in_=g1sb[:])
                    nc.sync.dma_start(out=gdump[:, NTT:], in_=g2sb[:])
                    hdump = nc.declare_dram_parameter(
                        "hdump", [P, HC * caps[0]], BF16, isOutput=True
                    )
                    nc.sync.dma_start(out=hdump[:], in_=hall[0][:])
                else:
                    tab = dp.tile([TROWS, D], BF16, tag="a_tab")

                comb_reads = []
                q_latest = {}

                def emit_combine(t0, L, late=False):
                    # combine L (1-2) adjacent token tiles. The table is
                    # interleaved (row = tile*256 + half*128 + p) so ONE read
                    # fetches both experts' rows; the per-tile math is a
                    # single fused DVE op t = b2*r + b1 (r = g2/g1, stored in
                    # g2sb) and y = Ln(t * g1) via the activation scale.
                    bg = cp.tile([P, L * 2 * D], BF16, tag="bg", bufs=3)
                    snap = dict(q_latest)
                    # reads on SP: their scatter-sem wait delays only the
                    # next iteration's input loads (slack there); on Pool it
                    # would block later scatters' desc-gen (measured 19us
                    # worse). EXCEPT the last groups (late=True): their waits
                    # resolve near iteration end, so on SP they pin the next
                    # iteration's gate loads — on Pool they block nothing
                    # (the next Pool op is ~60us away).
                    eng = nc.gpsimd if late else nc.sync
                    r1 = eng.dma_start(
                        out=bg[:].rearrange("p (l h j) -> p l h j", l=L, h=2),
                        in_=tab[t0 * 2 * P : (t0 + L) * 2 * P, :].rearrange(
                            "(l h p) j -> p l h j", l=L, h=2
                        ),
                    )
                    if late:
                        r1.ins.queue = f"qPoolDynamic{(t0 + 1) % NQ or ''}"
                    comb_reads.append((r1, snap))
                    sv = cps_.tile([P, L * D], FP32, tag="sv", bufs=2)
                    yv = cps_.tile([P, L * D], BF16, tag="yv", bufs=2)
                    for l in range(L):
                        t = t0 + l
                        nc.vector.scalar_tensor_tensor(
                            out=sv[:, l * D : (l + 1) * D],
                            in0=bg[:, (2 * l + 1) * D : (2 * l + 2) * D],
                            scalar=g2sb[:, t : t + 1],
                            in1=bg[:, 2 * l * D : (2 * l + 1) * D],
                            op0=mybir.AluOpType.mult,
                            op1=mybir.AluOpType.add,
                        )
                        nc.scalar.activation(
                            yv[:, l * D : (l + 1) * D],
                            sv[:, l * D : (l + 1) * D],
                            AF.Ln,
                            scale=g1sb[:, t : t + 1],
                        )
                    nc.scalar.dma_start(
                        out=y_d[t0 * P : (t0 + L) * P, :].rearrange(
                            "(l p) j -> p l j", l=L
                        ),
                        in_=yv[:].rearrange("p (l j) -> p l j", l=L),
                    )

                scatters = []
                ch = 0
                for k in range(E):
                    if k + 2 < E:
                        w2_pre[k + 2] = load_w(2, w2t_d, k + 2)
                    w2sb, b2sb = w2_pre.pop(k)
                    cap = caps[k]
                    hsb = hall[k]
                    for tt in range(ntts[k]):
                        m = min(P, cap - tt * P)
                        asb = ap_.tile([P, D], BF16, tag="a_sb")
                        ps2t = pp.tile([P, D], FP32, tag="fc2_ps", bufs=ps2)
                        nh = HC if "fc2" not in skip else 1
                        for h in range(nh):
                            nc.tensor.matmul(
                                ps2t[:m],
                                lhsT=hsb[:, h * cap + tt * P : h * cap + tt * P + m],
                                rhs=w2sb[:, h * D : (h + 1) * D],
                                start=(h == 0),
                                stop=(h == nh - 1 and not use_b2),
                            )
                        if use_b2:
                            nc.tensor.matmul(
                                ps2t[:m],
                                lhsT=ones1[:, :m],
                                rhs=b2sb[:],
                                start=False,
                                stop=True,
                            )
                        nc.scalar.activation(
                            asb[:m], ps2t[:m], AF.Exp, bias=fence[:m, 0:1]
                        )
                        if "scatter" not in skip:
                            mm = max(m, 2)
                            si = nc.gpsimd.indirect_dma_start(
                                out=tab[:],
                                out_offset=bass.IndirectOffsetOnAxis(
                                    ap=sidxsb[:mm, ch : ch + 1], axis=0
                                ),
                                in_=asb[:mm],
                                in_offset=None,
                            )
                            si.ins.queue = f"qPoolDynamic{(ch % NQ) or ''}"
                            q_latest[ch % NQ] = si
                            scatters.append(si)
                        ch += 1
                    if "tail" not in skip:
                        t = 0
                        while t < NTT:
                            if rsegs[t] == k:
                                L = 2 if (t + 1 < NTT and rsegs[t + 1] == k) else 1
                                emit_combine(t, L)
                                t += L
                            else:
                                t += 1
                # scatter destination rows are disjoint (injective dest map):
                # strip false WAW/WAR deps from the tracker's full-range AP so
                # scatters pipeline; combine reads then explicitly wait on the
                # latest scatter of every queue emitted before them.
                false_dep = {i.ins.name for i in scatters} | {
                    r.ins.name for r, _ in comb_reads
                }
                for si in scatters:
                    for nm in list(si.ins.sync_dependency_names()):
                        if nm in false_dep:
                            si.ins.try_remove_dependency(nm)
                dinfo = None
                for ri, snap in comb_reads:
                    have = set(ri.ins.sync_dependency_names())
                    if dinfo is None and have:
                        dinfo = ri.ins.get_dependency_info(next(iter(have)))
                    for si in snap.values():
                        if si.ins.name not in have:
                            ri.ins.add_dependency(si.ins.name, dinfo)

            if reps > 1:
                U = unroll
                while reps % U:
                    U -= 1
                with tc.For_i(0, reps // U, 1, staggered_reset=True):
                    for _u in range(U):
                        body()
            else:
                body()
            if timing:
                nc.sync.dma_start(out=yo_d[:], in_=dummy4[:])

    nc.compile()
    return nc


def _route(gate_feat, noise, w_gate, w_noise):
    """Host-side routing structure (fp32 numpy, matches jax top-k selection)."""
    clean = gate_feat @ w_gate
    stddev = np.logaddexp(gate_feat @ w_noise, 0.0) + np.float32(1e-2)
    logits = clean.astype(np.float32) + noise * stddev.astype(np.float32)
    top2 = np.argsort(-logits, axis=1, kind="stable")[:, :TOPK].astype(np.int32)
    return top2


def _prepare(x, gate_feat, noise, w_gate, w_noise, fc1_w, fc1_b, fc2_w, fc2_b):
    x = np.ascontiguousarray(x, dtype=np.float32)
    gate_feat = np.ascontiguousarray(gate_feat, dtype=np.float32)
    noise = np.ascontiguousarray(noise, dtype=np.float32)

    top2 = _route(gate_feat, noise, w_gate, w_noise)

    bf = ml_dtypes.bfloat16
    # w1l[e, p, h*512 + d*128 + q] = fc1_w[e, h*128+q, d*128+p]
    w1l_all = np.ascontiguousarray(
        fc1_w.reshape(E, HC, P, DC, P).transpose(0, 4, 1, 3, 2).reshape(E, P, DC * H)
    ).astype(bf)
    # w2l[e, p, h*512 + j] = fc2_w[e, j, h*128+p]
    w2l_all = np.ascontiguousarray(
        fc2_w.reshape(E, D, HC, P).transpose(0, 3, 2, 1).reshape(E, P, HC * D)
    ).astype(bf)
    b2_all = np.ascontiguousarray(fc2_b).astype(bf)
    wgwn = np.hstack([w_gate, w_noise]).astype(np.float32)
    # wgwn2[p, c*16+j] = wgwn[c*128+p, j]
    wgwn2 = np.ascontiguousarray(
        wgwn.reshape(FC, P, 2 * E).transpose(1, 0, 2).reshape(P, FC * 2 * E)
    ).astype(bf)

    core_meta = []
    for c in range(NC):
        t2 = top2[c * NS : (c + 1) * NS]
        cnt = np.bincount(t2.ravel(), minlength=E)
        order = np.argsort(-cnt, kind="stable").astype(np.int32)
        seg_of_expert = np.empty(E, dtype=np.int64)
        seg_of_expert[order] = np.arange(E)
        pair_seg = seg_of_expert[t2.ravel()]
        sort_idx = np.argsort(pair_seg, kind="stable")
        seg_counts = cnt[order]
        core_meta.append((t2, order, pair_seg, sort_idx, seg_counts))

    caps = np.max(np.stack([m[4] for m in core_meta]), axis=0)
    offs = np.concatenate([[0], np.cumsum(caps)]).astype(np.int64)
    R = int(offs[-1])
    ntts = [(int(c) + P - 1) // P for c in caps]
    NCH = sum(ntts)

    in_maps = []
    perms = []
    rsegs_cores = []
    for c in range(NC):
        t2, order, pair_seg, sort_idx, seg_counts = core_meta[c]
        pos_in_seg = np.arange(2 * NS) - np.concatenate([[0], np.cumsum(seg_counts)])[pair_seg[sort_idx]]
        rows_sorted = offs[pair_seg[sort_idx]] + pos_in_seg
        rows_of_pair = np.empty(2 * NS, dtype=np.int64)
        rows_of_pair[sort_idx] = rows_sorted

        ready = np.maximum(pair_seg[0::2], pair_seg[1::2])
        perm = np.argsort(ready, kind="stable")
        inv_perm = np.empty(NS, dtype=np.int64)
        inv_perm[perm] = np.arange(NS)
        rseg_core = ready[perm].reshape(NTT, P).max(axis=1)

        # interleaved table: row = tile*256 + half*128 + p
        dest = np.empty(R + P, dtype=np.int32)
        dest[:] = 2 * NS + (np.arange(R + P) % P)
        base = (inv_perm // P) * (2 * P) + (inv_perm % P)
        dest[rows_of_pair[0::2]] = base
        dest[rows_of_pair[1::2]] = base + P
        sidx = np.zeros((P, NCH), dtype=np.int32)
        chv = 0
        for k in range(E):
            for tt in range(ntts[k]):
                s = int(offs[k]) + tt * P
                sidx[:, chv] = dest[s : s + P]
                # rows past this segment's capacity would alias the NEXT
                # segment's dest entries — route them to the dump region
                # (scatters run concurrently, so aliasing is a race)
                m = min(P, int(caps[k]) - tt * P)
                if m < P:
                    sidx[m:, chv] = 2 * NS + np.arange(m, P)
                chv += 1

        tok_sorted = sort_idx // 2
        cols = np.zeros(R, dtype=np.int64)
        for k in range(E):
            s0 = int(np.concatenate([[0], np.cumsum(seg_counts)])[k])
            cnt_k = int(seg_counts[k])
            cols[offs[k] : offs[k] + cnt_k] = tok_sorted[s0 : s0 + cnt_k]
        x_loc = x[c * NS : (c + 1) * NS]
        # xl[p, DC*offs[k] + d*cap_k + t] = x_loc[cols_k[t], d*128+p]
        xl = np.zeros((P, DC * R), dtype=bf)
        for k in range(E):
            o0, cap_k = int(offs[k]), int(caps[k])
            seg = x_loc[cols[o0 : o0 + cap_k]]           # [cap, D]
            xl[:, DC * o0 : DC * (o0 + cap_k)] = (
                seg.reshape(cap_k, DC, P).transpose(2, 1, 0).reshape(P, DC * cap_k)
            )

        gf_loc = gate_feat[c * NS : (c + 1) * NS]
        # gft[p, t*1024 + c*128 + q] = gf_loc[perm[t*128+q], c*128+p]
        gfp = gf_loc[perm]                                # [NS, 2D]
        gft = np.ascontiguousarray(
            gfp.reshape(NTT, P, FC, P).transpose(3, 0, 2, 1).reshape(P, NTT * FC * P)
        ).astype(bf)

        ns_loc = noise[c * NS : (c + 1) * NS]
        nst = np.ascontiguousarray(
            ns_loc[perm].reshape(NTT, P, E).transpose(1, 0, 2).reshape(P, NTT * E)
        ).astype(np.float32)
        im = {
            "xl": np.ascontiguousarray(xl),
            "gft": gft,
            "nst": nst,
            "wgwn": wgwn2,
            "w1t": np.ascontiguousarray(w1l_all[order]),
            "w2t": np.ascontiguousarray(w2l_all[order]),
            "b2": np.ascontiguousarray(b2_all[order]),
            "sidx": sidx,
        }
        if np.any(fc1_b):
            im["b1"] = np.ascontiguousarray(
                fc1_b[order].reshape(E, HC, P).transpose(0, 2, 1)
            ).astype(np.float32)
        in_maps.append(im)
        perms.append(perm)
        rsegs_cores.append(rseg_core)

    rsegs = tuple(int(v) for v in np.max(np.stack(rsegs_cores), axis=0))
    return caps, rsegs, perms, in_maps


def kernel(x, gate_feat, noise, w_gate, w_noise, fc1_w, fc1_b, fc2_w, fc2_b,
           _reps=1):
    caps, rsegs, perms, in_maps = _prepare(
        x, gate_feat, noise, w_gate, w_noise, fc1_w, fc1_b, fc2_w, fc2_b
    )
    use_b2 = bool(np.any(np.asarray(fc2_b)))
    use_b1 = bool(np.any(np.asarray(fc1_b)))
    key = (tuple(int(v) for v in caps), rsegs, int(_reps), use_b2, use_b1)
    if key not in _nc_cache:
        _nc_cache[key] = _build_nc(caps, rsegs, reps=_reps, use_b2=use_b2,
                                   use_b1=use_b1)
    nc = _nc_cache[key]
    try:
        res = run_bass_kernel_spmd(nc, in_maps, core_ids=list(range(NC)))
    except Exception:
        res = run_bass_kernel_spmd(nc, in_maps, core_ids=list(range(NC)))
    y = np.empty((N, D), np.float32)
    for c in range(NC):
        y[c * NS : (c + 1) * NS][perms[c]] = res.results[c]["y"].astype(np.float32)
    return y


# revision 35
# speedup vs baseline: 1.4401x; 1.3964x over previous
"""MoE (noisy top-2-of-8 gating) Trainium2 kernel — v2.

Strategy: data-parallel over tokens (1024/core on 8 cores). Host computes
routing structure only; all FLOPs (gating values, expert MLPs, combine) run
on device. Tokens are permuted into 8 expert segments (experts sorted by
descending count; per-segment capacity = max count over cores so one SPMD
program serves all cores).

v2 changes vs the original (218994ns) kernel:
  * All DRAM operands are host-relaid so each logical load is 1-4 large
    DMAs (2KB+/partition lines): 8 gate-feature tile loads, 4 chunks per
    expert for fc1/fc2 weights, 1 per expert for x segments. HWDGE
    descriptor-generation drops from 218 ops (136us serialized) to ~90.
  * fc1 psums are built two h-chunks per 2-bank PSUM tile and drained by
    ONE gelu per pair: the ~390ns fixed ACT cost amortizes over 2x the
    elements, phase B stops being ACT-bound (78us -> ~53us of ACT).
  * The scatter table is interleaved (row = tile*256 + half*128 + p) so
    each combine group (1-2 token tiles sharing a readiness segment) is
    ONE strided read instead of 2-4; the combine math collapses to one
    fused DVE op per tile, t = b2*(g2/g1) + b1 via scalar_tensor_tensor,
    and y = Ln(t*g1) via the activation scale — the old mul/mul/add/Ln
    chain (4 ops, 2 round trips) becomes 2 ops.
  * Combine reads ride the SP HWDGE queue (measured faster than Pool,
    where their scatter-sem wait blocks later scatters' desc-gen) and y
    stores ride ACT; the Pool engine keeps only the 21 solo scatters.
    (Paired 2-column scatters compute wrong on HW — the DynamicAP index
    unroll order differs from CoreSim — so scatters stay one column per
    fc2 output tile. A 6-deep asb ring decouples scatter latency from
    the exp/PSUM drain.)
  * y is stored bf16 (host upcasts); adds ~2e-4 rel err, halves tail
    store time.

The Exp/Ln table-set chooser in bass is naive (picks the first set with
the function, thrashing between exp_and_others/natural_log on every
transition) — we point both at natural_log_exp_and_others (which
genuinely contains both) so each phase boundary is one table load.
"""

import numpy as np
import ml_dtypes

import concourse.bacc as bacc
import concourse.bass as bass
import concourse.mybir as mybir
import concourse.tile as tile
import concourse.hw_specs as hw_specs
from concourse.bass_utils import run_bass_kernel_spmd

BF16 = mybir.dt.bfloat16
FP32 = mybir.dt.float32
AF = mybir.ActivationFunctionType

N, D, H, E, TOPK = 8192, 512, 2048, 8, 2
NC = 8
NS = N // NC          # tokens per core
P = 128
NTT = NS // P         # token tiles per core (8)
DC = D // P           # d chunks (4)
HC = H // P           # hidden chunks (16)
HC2 = HC // 2         # fused gelu pairs (8)
FC = (2 * D) // P     # gate feature chunks (8)
NQ = 4                # SWDGE queues (hardware max)
WCH = 4               # weight load chunks per expert

_nc_cache: dict = {}
_act_tables_patched = [False]


def _patch_act_tables(arch: str):
    """Make Exp and Ln both resolve to natural_log_exp_and_others so the
    act-table fixpoint emits one load per phase instead of one per
    Exp<->Ln transition."""
    tabs = hw_specs.get_activation_tables(arch)
    if "natural_log_exp_and_others" in tabs:
        both = tabs["natural_log_exp_and_others"]
        if AF.Exp in both and AF.Ln in both:
            tabs["exp_and_others"].discard(AF.Exp)
            tabs["natural_log"].discard(AF.Ln)
    _act_tables_patched[0] = True


def _build_nc(caps, rsegs=(7,) * 8, reps=1, gelu_sub=False, timing=False,
              skip=(), wbufs=4, ps2=2, use_b2=True, use_b1=False, unroll=16,
              dbg=False, gsb_bufs=1):
    """Build the SPMD Bass program for per-segment capacities `caps`."""
    gelu_af = AF.Tanh if gelu_sub else AF.Gelu
    caps = tuple(int(c) for c in caps)
    R = sum(caps)
    offs = np.concatenate([[0], np.cumsum(caps)]).astype(int)
    ntts = [(c + P - 1) // P for c in caps]
    NCH = sum(ntts)              # fc2 output tiles == scatter chunk columns
    TROWS = 2 * NS + P           # interleaved token tables | dump rows

    nc = bacc.Bacc("TRN2", target_bir_lowering=False, debug=False,
                   num_swdge_queues=NQ)
    if not _act_tables_patched[0]:
        _patch_act_tables(nc.m.arch)

    if timing:
        def param(name, shape, dtype):
            return nc.dram_tensor(name, shape, dtype)
        dummy_d = nc.declare_dram_parameter("tdin", [1, 4], FP32, isOutput=False)
        y_d = nc.dram_tensor("y", [NS, D], BF16)
        yo_d = nc.declare_dram_parameter("yo", [1, 4], FP32, isOutput=True)
    else:
        def param(name, shape, dtype):
            return nc.declare_dram_parameter(name, shape, dtype, isOutput=False)
        y_d = nc.declare_dram_parameter("y", [NS, D], BF16, isOutput=True)

    xl_d = param("xl", [P, DC * R], BF16)
    gft_d = param("gft", [P, NTT * FC * P], BF16)
    nst_d = param("nst", [P, NTT * E], FP32)
    wgwn_d = param("wgwn", [P, FC * 2 * E], BF16)
    w1t_d = param("w1t", [E, P, DC * H], BF16)
    w2t_d = param("w2t", [E, P, HC * D], BF16)
    b2_d = param("b2", [E, D], BF16)
    b1_d = param("b1", [E, P, HC], FP32) if use_b1 else None
    sidx_d = param("sidx", [P, NCH], mybir.dt.int32)

    with tile.TileContext(nc) as tc:
        with (
            tc.tile_pool(name="const", bufs=1) as constp,
            tc.tile_pool(name="gate", bufs=1) as gatep,
            tc.tile_pool(name="hall", bufs=1) as hallp,
            tc.tile_pool(name="xpool", bufs=1) as xp,
            tc.tile_pool(name="wpool", bufs=wbufs) as wp,
            tc.tile_pool(name="apool", bufs=6) as ap_,
            tc.tile_pool(name="cpool", bufs=4) as cp,
            tc.tile_pool(name="cspool", bufs=3) as cps_,
            tc.tile_pool(name="psum", bufs=1, space="PSUM") as pp,
            tc.tile_pool(name="dram", bufs=1, space="DRAM") as dp,
        ):
            ones1 = constp.tile([1, P], BF16)
            nc.vector.memset(ones1[:], 1.0)
            dummy4 = constp.tile([1, 4], FP32)
            nc.vector.memset(dummy4[:], 0.0)

            def load_w(which, d_, k):
                wsb = wp.tile([P, DC * H], BF16, tag="w")
                step = (DC * H) // WCH
                for c in range(WCH if "wdma" not in skip else 1):
                    nc.sync.dma_start(
                        out=wsb[:, c * step : (c + 1) * step],
                        in_=d_[k, :, c * step : (c + 1) * step],
                    )
                if which == 2 and use_b2:
                    b2sb = wp.tile([1, D], BF16, tag="b2")
                    nc.sync.dma_start(out=b2sb[:], in_=b2_d[k][None, :])
                    return wsb, b2sb
                if which == 1 and use_b1:
                    b1sb = wp.tile([P, HC], FP32, tag="b1")
                    nc.sync.dma_start(out=b1sb[:], in_=b1_d[k])
                    return wsb, b1sb
                return wsb, None

            def load_x(k):
                cap = caps[k]
                o0 = int(offs[k])
                xk = xp.tile([P, DC * cap], BF16, tag=f"x{k}")
                nc.sync.dma_start(out=xk[:], in_=xl_d[:, DC * o0 : DC * (o0 + cap)])
                return xk

            def body(_i=None):
                # ---------- persistent loads (gating first) ----------
                gsb = gatep.tile([P, NTT * FC * P], BF16, tag="gsb",
                                 bufs=gsb_bufs)
                for t in range(NTT):
                    nc.sync.dma_start(
                        out=gsb[:, t * FC * P : (t + 1) * FC * P],
                        in_=gft_d[:, t * FC * P : (t + 1) * FC * P],
                    )
                wg2sb = gatep.tile([P, FC * 2 * E], BF16, tag="wg2sb")
                nc.sync.dma_start(out=wg2sb[:], in_=wgwn_d[:])
                nssb = gatep.tile([P, NTT * E], FP32, tag="nssb")
                nc.sync.dma_start(out=nssb[:], in_=nst_d[:])
                sidxsb = gatep.tile([P, NCH], mybir.dt.int32, tag="sidxsb")
                if timing:
                    nc.vector.memset(sidxsb[:], 0)
                else:
                    nc.sync.dma_start(out=sidxsb[:], in_=sidx_d[:])
                # x0 before the bulk of w1(0): the first fc1 psum pair needs
                # only w1 chunk 0 + x0, and x0 is the smaller transfer
                x_pre = {0: load_x(0)}
                w_pre = {0: load_w(1, w1t_d, 0)}
                x_pre[1] = load_x(1)

                # ---------- phase A: gating, token-major ----------
                g1sb = gatep.tile([P, NTT], FP32, tag="g1")
                g2sb = gatep.tile([P, NTT], FP32, tag="g2")
                if "gate" in skip:
                    nc.vector.memset(g1sb[:], 0.5)
                    nc.vector.memset(g2sb[:], 0.5)
                else:
                    W2E = 2 * E
                    clsb = gatep.tile([P, NTT * E], FP32, tag="clsb")
                    nssp = gatep.tile([P, NTT * E], FP32, tag="nssp")
                    for t in range(NTT):
                        gpsw = pp.tile([P, 2, 512], FP32, tag="fc1_ps", bufs=3)
                        gps = gpsw[:, 0, :W2E]
                        for c in range(FC):
                            nc.tensor.matmul(
                                gps[:],
                                lhsT=gsb[:, t * FC * P + c * P : t * FC * P + (c + 1) * P],
                                rhs=wg2sb[:, c * W2E : (c + 1) * W2E],
                                start=(c == 0),
                                stop=(c == FC - 1),
                            )
                        nc.vector.tensor_copy(
                            clsb[:, t * E : (t + 1) * E], gps[:, :E]
                        )
                        nc.vector.tensor_copy(
                            nssp[:, t * E : (t + 1) * E], gps[:, E:]
                        )
                    # stddev = softplus(noise logits) + 1e-2
                    nc.scalar.activation(nssp[:], nssp[:], AF.Exp)
                    nc.vector.tensor_scalar_add(nssp[:], nssp[:], 1.0)
                    nc.scalar.activation(nssp[:], nssp[:], AF.Ln)
                    nc.vector.tensor_scalar_add(nssp[:], nssp[:], 1e-2)
                    # logits = clean + noise * stddev; exp once — top-2 of
                    # exp(logits) == exp(top-2), so the gate softmax is pure
                    # DVE math after this
                    nc.vector.tensor_mul(nssp[:], nssp[:], nssb[:])
                    nc.vector.tensor_add(clsb[:], clsb[:], nssp[:])
                    nc.scalar.activation(clsb[:], clsb[:], AF.Exp)
                    # fence emitted BEFORE the g1/g2 math so gelus unblock as
                    # soon as the exp lands
                    gfence = gatep.tile([P, 1], FP32, tag="gfence")
                    nc.vector.tensor_scalar_mul(gfence[:], clsb[:, 0:1], 0.0)
                    mx8 = gatep.tile([P, NTT * E], FP32, tag="mx8")
                    t1g = gatep.tile([P, NTT], FP32, tag="t1g")
                    e1a = gatep.tile([P, NTT], FP32, tag="e1a")
                    e2a = gatep.tile([P, NTT], FP32, tag="e2a")
                    for t in range(NTT):
                        nc.vector.max(
                            out=mx8[:, t * E : (t + 1) * E],
                            in_=clsb[:, t * E : (t + 1) * E],
                        )
                        nc.vector.tensor_copy(
                            e1a[:, t : t + 1], mx8[:, t * E : t * E + 1]
                        )
                        nc.vector.tensor_copy(
                            e2a[:, t : t + 1], mx8[:, t * E + 1 : t * E + 2]
                        )
                    nc.vector.tensor_add(t1g[:], e1a[:], e2a[:])
                    nc.vector.reciprocal(t1g[:], t1g[:])
                    nc.vector.tensor_mul(g1sb[:], e1a[:], t1g[:])
                    # r = g2/g1 = e2/e1 for the fused combine (b1 + r*b2)
                    nc.vector.reciprocal(e1a[:], e1a[:])
                    nc.vector.tensor_mul(g2sb[:], e2a[:], e1a[:])

                # ---------- phase B: fc1 + gelu (fused pairs) ----------
                if "gate" in skip:
                    gfence = gatep.tile([P, 1], FP32, tag="gfence")
                    nc.vector.tensor_scalar_mul(gfence[:], g2sb[:, 0:1], 0.0)
                hall = []
                for k in range(E):
                    if k + 2 < E:
                        x_pre[k + 2] = load_x(k + 2)
                    if k + 1 < E:
                        w_pre[k + 1] = load_w(1, w1t_d, k + 1)
                    if k == E - 2:
                        w2_pre = {0: load_w(2, w2t_d, 0)}
                    if k == E - 1:
                        w2_pre[1] = load_w(2, w2t_d, 1)
                    w1sb, b1sb = w_pre.pop(k)
                    if use_b1:
                        b1f = wp.tile([P, HC], FP32, tag="b1f")
                        nc.vector.tensor_scalar_add(
                            b1f[:], b1sb[:], gfence[:, 0:1]
                        )
                    xk = x_pre.pop(k)
                    cap = caps[k]
                    hsb = hallp.tile([P, HC * cap], BF16, tag=f"h{k}")
                    for hp in range(HC2 if "fc1" not in skip else 1):
                        p2 = pp.tile([P, 2, 512], FP32, tag="fc1_ps", bufs=3)
                        for hh in range(2):
                            h = 2 * hp + hh
                            for d_ in range(DC):
                                nc.tensor.matmul(
                                    p2[:, hh, :cap],
                                    lhsT=w1sb[:, h * 512 + d_ * P : h * 512 + (d_ + 1) * P],
                                    rhs=xk[:, d_ * cap : (d_ + 1) * cap],
                                    start=(d_ == 0),
                                    stop=(d_ == DC - 1),
                                )
                        if "gelu" in skip:
                            nc.vector.tensor_copy(
                                hsb[:, 2 * hp * cap : (2 * hp + 2) * cap],
                                p2[:, :, :cap],
                            )
                        elif use_b1:
                            for hh in range(2):
                                h = 2 * hp + hh
                                nc.scalar.activation(
                                    hsb[:, h * cap : (h + 1) * cap],
                                    p2[:, hh, :cap],
                                    gelu_af,
                                    bias=b1f[:, h : h + 1],
                                )
                        else:
                            nc.scalar.activation(
                                hsb[:, 2 * hp * cap : (2 * hp + 2) * cap].rearrange(
                                    "p (two c) -> p two c", two=2
                                ),
                                p2[:, :, :cap],
                                gelu_af,
                                bias=gfence[:, 0:1],
                            )
                    hall.append(hsb)

                # ---------- phase C: fc2 + exp + scatter + combine ----------
                lastc = HC * caps[E - 1]
                fence = gatep.tile([P, 1], FP32, tag="fence")
                nc.vector.tensor_scalar_mul(
                    fence[:], hall[E - 1][:, lastc - 1 : lastc], 0.0
                )
                if dbg:
                    tab = nc.declare_dram_parameter(
                        "tabd", [TROWS, D], BF16, isOutput=True
                    )
                    gdump = nc.declare_dram_parameter(
                        "gdump", [P, 2 * NTT], FP32, isOutput=True
                    )
                    nc.sync.dma_start(out=gdump[:, :NTT], in_=g1sb[:])
                    nc.sync.dma_start(out=gdump[:, NTT:], in_=g2sb[:])
                    hdump = nc.declare_dram_parameter(
                        "hdump", [P, HC * caps[0]], BF16, isOutput=True
                    )
                    nc.sync.dma_start(out=hdump[:], in_=hall[0][:])
                else:
                    tab = dp.tile([TROWS, D], BF16, tag="a_tab")

                comb_reads = []
                q_latest = {}

                def emit_combine(t0, L, late=False):
                    # combine L (1-2) adjacent token tiles. The table is
                    # interleaved (row = tile*256 + half*128 + p) so ONE read
                    # fetches both experts' rows; the per-tile math is a
                    # single fused DVE op t = b2*r + b1 (r = g2/g1, stored in
                    # g2sb) and y = Ln(t * g1) via the activation scale.
                    bg = cp.tile([P, L * 2 * D], BF16, tag="bg", bufs=3)
                    snap = dict(q_latest)
                    # reads on SP: their scatter-sem wait delays only the
                    # next iteration's input loads (slack there); on Pool it
                    # would block later scatters' desc-gen (measured 19us
                    # worse). EXCEPT the last groups (late=True): their waits
                    # resolve near iteration end, so on SP they pin the next
                    # iteration's gate loads — on Pool they block nothing
                    # (the next Pool op is ~60us away).
                    eng = nc.gpsimd if late else nc.sync
                    r1 = eng.dma_start(
                        out=bg[:].rearrange("p (l h j) -> p l h j", l=L, h=2),
                        in_=tab[t0 * 2 * P : (t0 + L) * 2 * P, :].rearrange(
                            "(l h p) j -> p l h j", l=L, h=2
                        ),
                    )
                    if late:
                        r1.ins.queue = f"qPoolDynamic{(t0 + 1) % NQ or ''}"
                    comb_reads.append((r1, snap))
                    sv = cps_.tile([P, L * D], FP32, tag="sv", bufs=2)
                    yv = cps_.tile([P, L * D], BF16, tag="yv", bufs=2)
                    for l in range(L):
                        t = t0 + l
                        nc.vector.scalar_tensor_tensor(
                            out=sv[:, l * D : (l + 1) * D],
                            in0=bg[:, (2 * l + 1) * D : (2 * l + 2) * D],
                            scalar=g2sb[:, t : t + 1],
                            in1=bg[:, 2 * l * D : (2 * l + 1) * D],
                            op0=mybir.AluOpType.mult,
                            op1=mybir.AluOpType.add,
                        )
                        nc.scalar.activation(
                            yv[:, l * D : (l + 1) * D],
                            sv[:, l * D : (l + 1) * D],
                            AF.Ln,
                            scale=g1sb[:, t : t + 1],
                        )
                    nc.scalar.dma_start(
                        out=y_d[t0 * P : (t0 + L) * P, :].rearrange(
                            "(l p) j -> p l j", l=L
                        ),
                        in_=yv[:].rearrange("p (l j) -> p l j", l=L),
                    )

                scatters = []
                ch = 0
                for k in range(E):
                    if k + 2 < E:
                        w2_pre[k + 2] = load_w(2, w2t_d, k + 2)
                    w2sb, b2sb = w2_pre.pop(k)
                    cap = caps[k]
                    hsb = hall[k]
                    for tt in range(ntts[k]):
                        m = min(P, cap - tt * P)
                        asb = ap_.tile([P, D], BF16, tag="a_sb")
                        ps2t = pp.tile([P, D], FP32, tag="fc2_ps", bufs=ps2)
                        nh = HC if "fc2" not in skip else 1
                        for h in range(nh):
                            nc.tensor.matmul(
                                ps2t[:m],
                                lhsT=hsb[:, h * cap + tt * P : h * cap + tt * P + m],
                                rhs=w2sb[:, h * D : (h + 1) * D],
                                start=(h == 0),
                                stop=(h == nh - 1 and not use_b2),
                            )
                        if use_b2:
                            nc.tensor.matmul(
                                ps2t[:m],
                                lhsT=ones1[:, :m],
                                rhs=b2sb[:],
                                start=False,
                                stop=True,
                            )
                        nc.scalar.activation(
                            asb[:m], ps2t[:m], AF.Exp, bias=fence[:m, 0:1]
                        )
                        if "scatter" not in skip:
                            mm = max(m, 2)
                            si = nc.gpsimd.indirect_dma_start(
                                out=tab[:],
                                out_offset=bass.IndirectOffsetOnAxis(
                                    ap=sidxsb[:mm, ch : ch + 1], axis=0
                                ),
                                in_=asb[:mm],
                                in_offset=None,
                            )
                            si.ins.queue = f"qPoolDynamic{(ch % NQ) or ''}"
                            q_latest[ch % NQ] = si
                            scatters.append(si)
                        ch += 1
                    if "tail" not in skip:
                        t = 0
                        while t < NTT:
                            if rsegs[t] == k:
                                L = 2 if (t + 1 < NTT and rsegs[t + 1] == k) else 1
                                emit_combine(t, L)
                                t += L
                            else:
                                t += 1
                # scatter destination rows are disjoint (injective dest map):
                # strip false WAW/WAR deps from the tracker's full-range AP so
                # scatters pipeline; combine reads then explicitly wait on the
                # latest scatter of every queue emitted before them.
                false_dep = {i.ins.name for i in scatters} | {
                    r.ins.name for r, _ in comb_reads
                }
                for si in scatters:
                    for nm in list(si.ins.sync_dependency_names()):
                        if nm in false_dep:
                            si.ins.try_remove_dependency(nm)
                dinfo = None
                for ri, snap in comb_reads:
                    have = set(ri.ins.sync_dependency_names())
                    if dinfo is None and have:
                        dinfo = ri.ins.get_dependency_info(next(iter(have)))
                    for si in snap.values():
                        if si.ins.name not in have:
                            ri.ins.add_dependency(si.ins.name, dinfo)

            if reps > 1:
                U = unroll
                while reps % U:
                    U -= 1
                with tc.For_i(0, reps // U, 1, staggered_reset=True):
                    for _u in range(U):
                        body()
            else:
                body()
            if timing:
                nc.sync.dma_start(out=yo_d[:], in_=dummy4[:])

    nc.compile()
    return nc


def _route(gate_feat, noise, w_gate, w_noise):
    """Host-side routing structure (fp32 numpy, matches jax top-k selection)."""
    clean = gate_feat @ w_gate
    stddev = np.logaddexp(gate_feat @ w_noise, 0.0) + np.float32(1e-2)
    logits = clean.astype(np.float32) + noise * stddev.astype(np.float32)
    top2 = np.argsort(-logits, axis=1, kind="stable")[:, :TOPK].astype(np.int32)
    return top2


def _prepare(x, gate_feat, noise, w_gate, w_noise, fc1_w, fc1_b, fc2_w, fc2_b):
    x = np.ascontiguousarray(x, dtype=np.float32)
    gate_feat = np.ascontiguousarray(gate_feat, dtype=np.float32)
    noise = np.ascontiguousarray(noise, dtype=np.float32)

    top2 = _route(gate_feat, noise, w_gate, w_noise)

    bf = ml_dtypes.bfloat16
    # w1l[e, p, h*512 + d*128 + q] = fc1_w[e, h*128+q, d*128+p]
    w1l_all = np.ascontiguousarray(
        fc1_w.reshape(E, HC, P, DC, P).transpose(0, 4, 1, 3, 2).reshape(E, P, DC * H)
    ).astype(bf)
    # w2l[e, p, h*512 + j] = fc2_w[e, j, h*128+p]
    w2l_all = np.ascontiguousarray(
        fc2_w.reshape(E, D, HC, P).transpose(0, 3, 2, 1).reshape(E, P, HC * D)
    ).astype(bf)
    b2_all = np.ascontiguousarray(fc2_b).astype(bf)
    wgwn = np.hstack([w_gate, w_noise]).astype(np.float32)
    # wgwn2[p, c*16+j] = wgwn[c*128+p, j]
    wgwn2 = np.ascontiguousarray(
        wgwn.reshape(FC, P, 2 * E).transpose(1, 0, 2).reshape(P, FC * 2 * E)
    ).astype(bf)

    core_meta = []
    for c in range(NC):
        t2 = top2[c * NS : (c + 1) * NS]
        cnt = np.bincount(t2.ravel(), minlength=E)
        order = np.argsort(-cnt, kind="stable").astype(np.int32)
        seg_of_expert = np.empty(E, dtype=np.int64)
        seg_of_expert[order] = np.arange(E)
        pair_seg = seg_of_expert[t2.ravel()]
        sort_idx = np.argsort(pair_seg, kind="stable")
        seg_counts = cnt[order]
        core_meta.append((t2, order, pair_seg, sort_idx, seg_counts))

    caps = np.max(np.stack([m[4] for m in core_meta]), axis=0)
    offs = np.concatenate([[0], np.cumsum(caps)]).astype(np.int64)
    R = int(offs[-1])
    ntts = [(int(c) + P - 1) // P for c in caps]
    NCH = sum(ntts)

    in_maps = []
    perms = []
    rsegs_cores = []
    for c in range(NC):
        t2, order, pair_seg, sort_idx, seg_counts = core_meta[c]
        pos_in_seg = np.arange(2 * NS) - np.concatenate([[0], np.cumsum(seg_counts)])[pair_seg[sort_idx]]
        rows_sorted = offs[pair_seg[sort_idx]] + pos_in_seg
        rows_of_pair = np.empty(2 * NS, dtype=np.int64)
        rows_of_pair[sort_idx] = rows_sorted

        ready = np.maximum(pair_seg[0::2], pair_seg[1::2])
        perm = np.argsort(ready, kind="stable")
        inv_perm = np.empty(NS, dtype=np.int64)
        inv_perm[perm] = np.arange(NS)
        rseg_core = ready[perm].reshape(NTT, P).max(axis=1)

        # interleaved table: row = tile*256 + half*128 + p
        dest = np.empty(R + P, dtype=np.int32)
        dest[:] = 2 * NS + (np.arange(R + P) % P)
        base = (inv_perm // P) * (2 * P) + (inv_perm % P)
        dest[rows_of_pair[0::2]] = base
        dest[rows_of_pair[1::2]] = base + P
        sidx = np.zeros((P, NCH), dtype=np.int32)
        chv = 0
        for k in range(E):
            for tt in range(ntts[k]):
                s = int(offs[k]) + tt * P
                sidx[:, chv] = dest[s : s + P]
                # rows past this segment's capacity would alias the NEXT
                # segment's dest entries — route them to the dump region
                # (scatters run concurrently, so aliasing is a race)
                m = min(P, int(caps[k]) - tt * P)
                if m < P:
                    sidx[m:, chv] = 2 * NS + np.arange(m, P)
                chv += 1

        tok_sorted = sort_idx // 2
        cols = np.zeros(R, dtype=np.int64)
        for k in range(E):
            s0 = int(np.concatenate([[0], np.cumsum(seg_counts)])[k])
            cnt_k = int(seg_counts[k])
            cols[offs[k] : offs[k] + cnt_k] = tok_sorted[s0 : s0 + cnt_k]
        x_loc = x[c * NS : (c + 1) * NS]
        # xl[p, DC*offs[k] + d*cap_k + t] = x_loc[cols_k[t], d*128+p]
        xl = np.zeros((P, DC * R), dtype=bf)
        for k in range(E):
            o0, cap_k = int(offs[k]), int(caps[k])
            seg = x_loc[cols[o0 : o0 + cap_k]]           # [cap, D]
            xl[:, DC * o0 : DC * (o0 + cap_k)] = (
                seg.reshape(cap_k, DC, P).transpose(2, 1, 0).reshape(P, DC * cap_k)
            )

        gf_loc = gate_feat[c * NS : (c + 1) * NS]
        # gft[p, t*1024 + c*128 + q] = gf_loc[perm[t*128+q], c*128+p]
        gfp = gf_loc[perm]                                # [NS, 2D]
        gft = np.ascontiguousarray(
            gfp.reshape(NTT, P, FC, P).transpose(3, 0, 2, 1).reshape(P, NTT * FC * P)
        ).astype(bf)

        ns_loc = noise[c * NS : (c + 1) * NS]
        nst = np.ascontiguousarray(
            ns_loc[perm].reshape(NTT, P, E).transpose(1, 0, 2).reshape(P, NTT * E)
        ).astype(np.float32)
        im = {
            "xl": np.ascontiguousarray(xl),
            "gft": gft,
            "nst": nst,
            "wgwn": wgwn2,
            "w1t": np.ascontiguousarray(w1l_all[order]),
            "w2t": np.ascontiguousarray(w2l_all[order]),
            "b2": np.ascontiguousarray(b2_all[order]),
            "sidx": sidx,
        }
        if np.any(fc1_b):
            im["b1"] = np.ascontiguousarray(
                fc1_b[order].reshape(E, HC, P).transpose(0, 2, 1)
            ).astype(np.float32)
        in_maps.append(im)
        perms.append(perm)
        rsegs_cores.append(rseg_core)

    rsegs = tuple(int(v) for v in np.max(np.stack(rsegs_cores), axis=0))
    return caps, rsegs, perms, in_maps


def kernel(x, gate_feat, noise, w_gate, w_noise, fc1_w, fc1_b, fc2_w, fc2_b,
           _reps=1):
    caps, rsegs, perms, in_maps = _prepare(
        x, gate_feat, noise, w_gate, w_noise, fc1_w, fc1_b, fc2_w, fc2_b
    )
    use_b2 = bool(np.any(np.asarray(fc2_b)))
    use_b1 = bool(np.any(np.asarray(fc1_b)))
    key = (tuple(int(v) for v in caps), rsegs, int(_reps), use_b2, use_b1)
    if key not in _nc_cache:
        _nc_cache[key] = _build_nc(caps, rsegs, reps=_reps, use_b2=use_b2,
                                   use_b1=use_b1)
    nc = _nc_cache[key]
    try:
        res = run_bass_kernel_spmd(nc, in_maps, core_ids=list(range(NC)))
    except Exception:
        res = run_bass_kernel_spmd(nc, in_maps, core_ids=list(range(NC)))
    y = np.empty((N, D), np.float32)
    for c in range(NC):
        y[c * NS : (c + 1) * NS][perms[c]] = res.results[c]["y"].astype(np.float32)
    return y
